# revision 1
# baseline (speedup 1.0000x reference)
"""Trainium2 Bass kernel for nn_DistiledMultiheadAttention_76476187673064.

Sliding-window (W=32) single-query attention over ragged sequences with a
learned pre-context buffer, plus input/output projections.

Strategy (8 NeuronCores, data-parallel over flat tokens):
  - Each core owns 512 tokens; kv for a 31-token halo is recomputed locally
    (plus one masked pad column), so no collectives are needed.
  - Host passes transposed weights/activations so every matmul's contraction
    dim lands on SBUF partitions with zero on-device transposes:
      * K/Q projections feature-major (k_T, q_T: [feat, tok])
      * V projection token-major, augmented with a ones column per head
        (so PV emits per-head softmax sums for free)
      * QK logits ctx-major [ctx, tok]; band+segment+buffer masking via a
        host-precomputed additive mask; exp without max-subtraction
        (logits are bounded); PV ctx-major -> o feature-major
      * softmax normalization: reciprocal of the sums row, broadcast across
        partitions with a rank-1 matmul (ones ⊗ r), applied as the PSUM
        eviction multiply
      * biases via rank-1 matmuls accumulated into PSUM (ones ⊗ bias-row)
  - All matmuls run as float32r (full-rate fp32 storage) via AP bitcast.
"""
import math
import sys

sys.path.insert(0, "/opt/trn_rl_repo")

import numpy as np

# ---------------------------------------------------------------- constants
T = 4096
E = 1024
KD = 512          # key dim
H = 16            # heads
W = 32            # window
DK = KD // H      # 32
DV = E // H       # 64
B = 8
MAXL = 768
N_CORES = 8
SHARD = T // N_CORES          # 512 tokens per core
HALO = W - 1                  # 31
NTOK = SHARD + HALO + 1       # 544 token columns incl. halo + 1 pad
TB = 256                      # attention token block
NB = SHARD // TB              # 2 blocks per core
CTXF = 1024                   # logits free: band [128,512] + tail [32,256] + buf [32,256]
NEG = -30000.0

_CACHE = {}


# ------------------------------------------------------------- tile patches
def _apply_tile_patches():
    """This container's walrus only supports ONE sync-wait per instruction;
    redistribute extra Tile-assigned waits onto single-wait InstNoOp carriers."""
    import concourse.mybir as mybir
    import concourse.tile as tile
    from concourse.vector_clock import ScopedClock

    if getattr(tile.TileContext, "_wait_split_patched", False):
        return
    orig_commit = tile.TileContext._commit_and_lower

    def commit_split(self, inst, original_block, old_bb_map, bb_to_exit_bb):
        si = getattr(inst, "sync_info", None)
        if si is not None and si.on_wait and len(si.on_wait) > 1:
            engine = inst.engine
            if engine is not None and engine != mybir.EngineType.Unassigned:
                waits = list(si.on_wait)
                si.on_wait = waits[-1:]
                for w in waits[:-1]:
                    noop = mybir.InstNoOp(
                        name=self.nc.get_next_instruction_name(),
                        sync_info=mybir.SyncInfo(on_wait=[w], on_update=[]),
                        bass_nofuse=True,
                        engine=engine,
                        text_hint="wait_split",
                    )
                    orig_commit(self, noop, original_block, old_bb_map, bb_to_exit_bb)
        return orig_commit(self, inst, original_block, old_bb_map, bb_to_exit_bb)

    def drain_and_barrier(self, tick_clock, wait_clock):
        drain_inst = self.nc.sync.drain()
        wait_clock.add_sem_waits(
            drain_inst.ins, ScopedClock({None: tick_clock.global_clock})
        )
        si = drain_inst.ins.sync_info
        if si is not None and si.on_wait and len(si.on_wait) > 1:
            waits = list(si.on_wait)
            si.on_wait = waits[:1]
            for w in waits[1:]:
                nop = self.nc.sync.nop(nofuse=True)
                nsi = nop.ins.sync_info
                if nsi is None:
                    nop.ins.sync_info = mybir.SyncInfo(on_wait=[w], on_update=[])
                else:
                    nsi.on_wait = list(nsi.on_wait or []) + [w]
        self.nc.all_engine_barrier()
        assert self.sems is not None
        popped = self.nc._tile_sem_poison_stack.pop()
        assert popped is self._sem_poison
        self.nc.clear_and_free_semaphores(list(self.sems.allocated().values()))
        self.nc.all_engine_barrier()

    tile.TileContext._commit_and_lower = commit_split
    tile.TileContext._drain_and_barrier = drain_and_barrier
    tile.TileContext._wait_split_patched = True


def _act_recip(nc, out, in_):
    """ScalarE LUT reciprocal (bass gates ActivationFunctionType.Reciprocal
    behind a ValueError for accuracy; softmax denominators tolerate it —
    verified end-to-end against the reference)."""
    import concourse.mybir as mybir

    eng = nc.scalar
    inputs = [eng.lower_ap(in_)]
    for arg in (0.0, 1.0, 0.0):  # bias, scale, alpha
        inputs.append(mybir.ImmediateValue(dtype=mybir.dt.float32, value=arg))
    return eng.add_instruction(
        mybir.InstActivation(
            name=nc.get_next_instruction_name(),
            func=mybir.ActivationFunctionType.Reciprocal,
            ins=inputs,
            outs=[eng.lower_ap(out)],
        )
    )


# ------------------------------------------------------------- device build
def _build_nc(with_bias=True):
    import concourse.bass as bass
    import concourse.mybir as mybir
    import concourse.tile as tile

    import bass_rust

    _apply_tile_patches()
    f32 = mybir.dt.float32
    f32r = mybir.dt.float32r

    nc = bass.Bass()
    d_xT = nc.dram_tensor("xT", [E, NTOK], f32r, kind="ExternalInput")
    d_wk = nc.dram_tensor("wk", [E, KD], f32r, kind="ExternalInput")
    d_wv = nc.dram_tensor("wv", [E, H * 65], f32r, kind="ExternalInput")
    d_wq = nc.dram_tensor("wq", [E, KD], f32r, kind="ExternalInput")
    d_wp = nc.dram_tensor("wp", [E, E], f32r, kind="ExternalInput")
    d_bk = nc.dram_tensor("bk", [128, 4], f32, kind="ExternalInput")
    d_bq = nc.dram_tensor("bq", [128, 4], f32, kind="ExternalInput")
    d_bv = nc.dram_tensor("bv", [1, H * 65], f32r, kind="ExternalInput")
    d_bp = nc.dram_tensor("bp", [1, E], f32r, kind="ExternalInput")
    d_ones = nc.dram_tensor("ones", [1, 128], f32r, kind="ExternalInput")
    d_kbufT = nc.dram_tensor("kbufT", [KD, 32], f32r, kind="ExternalInput")
    d_vbuf = nc.dram_tensor("vbuf", [32, H * 65], f32r, kind="ExternalInput")
    d_mask = nc.dram_tensor("mask", [NB, 128, CTXF], f32, kind="ExternalInput")
    d_y = nc.dram_tensor("yout", [SHARD, E], f32, kind="ExternalOutput")

    with tile.TileContext(nc) as tc, nc.allow_low_precision(
        reason="f32r matmul operands; fp32 PSUM accumulation throughout"
    ):
        with (
            tc.tile_pool(name="const", bufs=1) as const_pool,
            tc.tile_pool(name="x", bufs=1) as x_pool,
            tc.tile_pool(name="kqv", bufs=1) as kqv_pool,
            tc.tile_pool(name="wsmall", bufs=3) as ws_pool,
            tc.tile_pool(name="wbig", bufs=2) as wb_pool,
            tc.tile_pool(name="exp", bufs=10) as exp_pool,
            tc.tile_pool(name="rrow", bufs=4) as r_pool,
            tc.tile_pool(name="out", bufs=3) as out_pool,
        ):
            # ---- constants
            ones = const_pool.tile([1, 128], f32r)
            nc.sync.dma_start(ones[:], d_ones[:])
            bk_sb = const_pool.tile([128, 4], f32)
            nc.sync.dma_start(bk_sb[:], d_bk[:])
            bq_sb = const_pool.tile([128, 4], f32)
            nc.sync.dma_start(bq_sb[:], d_bq[:])
            bv_sb = const_pool.tile([1, H * 65], f32r)
            nc.sync.dma_start(bv_sb[:], d_bv[:])
            bp_sb = const_pool.tile([1, E], f32r)
            nc.sync.dma_start(bp_sb[:], d_bp[:])
            kbuf_sb = const_pool.tile([128, 4, 32], f32r)
            nc.sync.dma_start(
                kbuf_sb[:], d_kbufT[:].rearrange("(m p) c -> p m c", p=128)
            )
            vbuf_sb = const_pool.tile([32, H * 65], f32r)
            nc.sync.dma_start(vbuf_sb[:], d_vbuf[:])
            mask_sb = const_pool.tile([128, NB, CTXF], f32)
            nc.sync.dma_start(
                mask_sb[:], d_mask[:].rearrange("b p f -> p b f")
            )

            # ---- x (feature-major, all tokens incl. halo+pad)
            xT = x_pool.tile([128, 8, NTOK], f32r)
            nc.sync.dma_start(xT[:], d_xT[:].rearrange("(c p) t -> p c t", p=128))

            # ---- persistent activations
            kT = kqv_pool.tile([128, 4, NTOK], f32r)   # K feature-major
            qT = kqv_pool.tile([128, 4, SHARD], f32r)  # Q feature-major (scaled)
            vA = kqv_pool.tile([128, 5, H * 65], f32r)  # V token-major + ones col
            oT = kqv_pool.tile([128, 8, SHARD], f32r)  # attention out feature-major

            with tc.tile_pool(name="pp1", bufs=2, space="PSUM") as pp1:
                # K projection: kT[:, m, :] = wk[:, mchunk].T @ xT (+ bk)
                for m in range(4):
                    wkt = ws_pool.tile([128, 8, 128], f32r, tag="wkq")
                    nc.sync.dma_start(
                        wkt[:],
                        d_wk[:, m * 128:(m + 1) * 128].rearrange(
                            "(c p) f -> p c f", p=128
                        ),
                    )
                    pa = pp1.tile([128, 512], f32, tag="pa")
                    pb = pp1.tile([128, 32], f32, tag="pb")
                    for e in range(8):
                        nc.tensor.matmul(
                            pa[:], wkt[:, e, :], xT[:, e, 0:512],
                            start=(e == 0), stop=(e == 7),
                        )
                        nc.tensor.matmul(
                            pb[:], wkt[:, e, :], xT[:, e, 512:NTOK],
                            start=(e == 0), stop=(e == 7),
                        )
                    if with_bias:
                        nc.vector.tensor_scalar_add(kT[:, m, 0:512], pa[:], bk_sb[:, m:m + 1])
                        nc.vector.tensor_scalar_add(kT[:, m, 512:NTOK], pb[:], bk_sb[:, m:m + 1])
                    else:
                        nc.vector.tensor_copy(kT[:, m, 0:512], pa[:])
                        nc.vector.tensor_copy(kT[:, m, 512:NTOK], pb[:])

                # Q projection (tokens only, no halo): qT = wq.T @ xT[:, 31:543] (+ bq)
                for m in range(4):
                    wqt = ws_pool.tile([128, 8, 128], f32r, tag="wkq")
                    nc.sync.dma_start(
                        wqt[:],
                        d_wq[:, m * 128:(m + 1) * 128].rearrange(
                            "(c p) f -> p c f", p=128
                        ),
                    )
                    pa = pp1.tile([128, 512], f32, tag="pa")
                    for e in range(8):
                        nc.tensor.matmul(
                            pa[:], wqt[:, e, :], xT[:, e, HALO:HALO + SHARD],
                            start=(e == 0), stop=(e == 7),
                        )
                    if with_bias:
                        nc.vector.tensor_scalar_add(qT[:, m, :], pa[:], bq_sb[:, m:m + 1])
                    else:
                        nc.vector.tensor_copy(qT[:, m, :], pa[:])

                # V projection token-major (wv pre-augmented with ones cols):
                # vA[tok, h*65:h*65+65] = [x @ Wv_h.T + bv_h | 1]
                tok_sizes = [128, 128, 128, 128, 32]
                for f in range(4):
                    wvt = wb_pool.tile([128, 8, 260], f32r, tag="wv")
                    nc.sync.dma_start(
                        wvt[:],
                        d_wv[:, f * 260:(f + 1) * 260].rearrange(
                            "(c p) f2 -> p c f2", p=128
                        ),
                    )
                    for i in range(5):
                        mt = tok_sizes[i]
                        pa = pp1.tile([128, 260], f32, tag="pv")
                        for e in range(8):
                            nc.tensor.matmul(
                                pa[0:mt, :],
                                xT[:, e, i * 128:i * 128 + mt],
                                wvt[:, e, :],
                                start=(e == 0), stop=(e == 7 and not with_bias),
                            )
                        if with_bias:
                            nc.tensor.matmul(
                                pa[0:mt, :], ones[0:1, 0:mt],
                                bv_sb[0:1, f * 260:(f + 1) * 260],
                                start=False, stop=True,
                            )
                        nc.vector.tensor_copy(
                            vA[0:mt, i, f * 260:(f + 1) * 260], pa[0:mt, :]
                        )
                        if not with_bias:
                            # ones columns via strided add (psum zeros there)
                            ov_view = vA[0:mt, i, f * 260:(f + 1) * 260].rearrange(
                                "p (h c) -> p h c", c=65
                            )[:, :, 64:65]
                            nc.vector.tensor_scalar_add(ov_view, ov_view, 1.0)

            # ---- attention
            with (
                tc.tile_pool(name="plg", bufs=2, space="PSUM") as plg,
                tc.tile_pool(name="pov", bufs=2, space="PSUM") as pov,
                tc.tile_pool(name="prb", bufs=2, space="PSUM") as prb,
            ):
                GRP = 8
                prev_last_recip = None
                for b in range(NB):
                    base = b * TB
                    for g in range(H // GRP):
                        exs = []
                        first_exp = None
                        # phase A: QK + mask + exp for the whole group
                        # (one Exp table residency; dense PE matmul burst)
                        for hh in range(GRP):
                            h = g * GRP + hh
                            ro = (h % 4) * 32
                            ht = h // 4
                            lg = plg.tile([128, CTXF], f32)
                            qh = qT[ro:ro + 32, ht, base:base + TB]
                            nc.tensor.matmul(
                                lg[:, 0:256], kT[ro:ro + 32, ht, base:base + 128],
                                qh, start=True, stop=True, tile_position=(ro, 0),
                            )
                            nc.tensor.matmul(
                                lg[:, 256:512], kT[ro:ro + 32, ht, base + 128:base + 256],
                                qh, start=True, stop=True, tile_position=(ro, 0),
                            )
                            nc.tensor.matmul(
                                lg[0:32, 512:768], kT[ro:ro + 32, ht, base + 256:base + 288],
                                qh, start=True, stop=True, tile_position=(ro, 0),
                            )
                            nc.tensor.matmul(
                                lg[0:32, 768:1024], kbuf_sb[ro:ro + 32, ht, :],
                                qh, start=True, stop=True, tile_position=(ro, 0),
                            )
                            nc.vector.tensor_tensor(
                                lg[:, 0:512], lg[:, 0:512], mask_sb[:, b, 0:512],
                                mybir.AluOpType.add,
                            )
                            nc.vector.tensor_tensor(
                                lg[0:32, 512:1024], lg[0:32, 512:1024],
                                mask_sb[0:32, b, 512:1024], mybir.AluOpType.add,
                            )
                            ex = exp_pool.tile([128, CTXF], f32r)
                            e1 = nc.scalar.activation(
                                ex[:, 0:512], lg[:, 0:512],
                                mybir.ActivationFunctionType.Exp,
                            )
                            if first_exp is None:
                                first_exp = e1
                                if prev_last_recip is not None:
                                    # keep ACT's Exp/Reciprocal LUT loads
                                    # batched per phase (ordering-only dep)
                                    bass_rust.add_dep_helper(
                                        e1.ins, prev_last_recip.ins,
                                        sync=False,
                                        reason="ACT table residency batching",
                                    )
                            nc.scalar.activation(
                                ex[0:32, 512:1024], lg[0:32, 512:1024],
                                mybir.ActivationFunctionType.Exp,
                            )
                            exs.append(ex)
                        # phase B: PV + normalization for the group
                        # (one Reciprocal table residency)
                        for hh in range(GRP):
                            h = g * GRP + hh
                            ex = exs[hh]
                            ov = pov.tile([128, TB], f32)
                            hc = h * 65
                            nc.tensor.matmul(
                                ov[0:65, :], vA[:, 2 * b, hc:hc + 65],
                                ex[:, 0:256], start=True, stop=False,
                            )
                            nc.tensor.matmul(
                                ov[0:65, :], vA[:, 2 * b + 1, hc:hc + 65],
                                ex[:, 256:512], start=False, stop=False,
                            )
                            nc.tensor.matmul(
                                ov[0:65, :], vA[0:32, 2 * b + 2, hc:hc + 65],
                                ex[0:32, 512:768], start=False, stop=False,
                            )
                            nc.tensor.matmul(
                                ov[0:65, :], vbuf_sb[:, hc:hc + 65],
                                ex[0:32, 768:1024], start=False, stop=True,
                            )
                            sr = r_pool.tile([1, TB], f32r)
                            prev_last_recip = _act_recip(nc, sr[:], ov[64:65, :])
                            rb = prb.tile([64, TB], f32)
                            nc.tensor.matmul(
                                rb[:], ones[0:1, 0:64], sr[:],
                                start=True, stop=True,
                            )
                            od = oT[(h % 2) * 64:(h % 2) * 64 + 64, h // 2,
                                    base:base + TB]
                            nc.vector.tensor_copy(od, ov[0:64, :])
                            nc.vector.tensor_tensor(
                                od, od, rb[:], mybir.AluOpType.mult,
                            )

            # ---- output projection: y[tok, :] = oT.T @ wp (+ bp)
            with tc.tile_pool(name="pp3", bufs=2, space="PSUM") as pp3:
                for f in range(2):
                    wpt = wb_pool.tile([128, 8, 512], f32r, tag="wbig")
                    nc.sync.dma_start(
                        wpt[:],
                        d_wp[:, f * 512:(f + 1) * 512].rearrange(
                            "(c p) f2 -> p c f2", p=128
                        ),
                    )
                    for m in range(4):
                        pa = pp3.tile([128, 512], f32)
                        for c in range(8):
                            nc.tensor.matmul(
                                pa[:], oT[:, c, m * 128:(m + 1) * 128],
                                wpt[:, c, :], start=(c == 0),
                                stop=(c == 7 and not with_bias),
                            )
                        if with_bias:
                            nc.tensor.matmul(
                                pa[:], ones[0:1, 0:128],
                                bp_sb[0:1, f * 512:(f + 1) * 512],
                                start=False, stop=True,
                            )
                        ot = out_pool.tile([128, 512], f32)
                        nc.vector.tensor_copy(ot[:], pa[:])
                        nc.sync.dma_start(
                            d_y[m * 128:(m + 1) * 128, f * 512:(f + 1) * 512], ot[:]
                        )
    return nc


def _get_runner(with_bias=True):
    key = ("runner", with_bias)
    if key in _CACHE:
        return _CACHE[key]
    import jax
    import concourse.mybir as mybir
    from concourse import bass2jax
    from jax.sharding import Mesh, PartitionSpec
    from jax.experimental.shard_map import shard_map

    nc = _build_nc(with_bias)
    bass2jax.install_neuronx_cc_hook()
    partition_name = nc.partition_id_tensor.name if nc.partition_id_tensor else None
    in_names, out_names, out_avals, out_shapes = [], [], [], []
    for alloc in nc.m.functions[0].allocations:
        if not isinstance(alloc, mybir.MemoryLocationSet):
            continue
        name = alloc.memorylocations[0].name
        if alloc.kind == "ExternalInput":
            if name != partition_name:
                in_names.append(name)
        elif alloc.kind == "ExternalOutput":
            shape = tuple(alloc.tensor_shape)
            dtype = mybir.dt.np(alloc.dtype)
            out_names.append(name)
            out_avals.append(jax.core.ShapedArray(shape, dtype))
            out_shapes.append((shape, dtype))
    n_params = len(in_names)
    n_outs = len(out_avals)
    all_in_names = in_names + out_names + ([partition_name] if partition_name else [])
    donate = tuple(range(n_params, n_params + n_outs))

    def _body(*args):
        operands = list(args)
        if partition_name is not None:
            operands.append(bass2jax.partition_id_tensor())
        outs = bass2jax._bass_exec_p.bind(
            *operands,
            out_avals=tuple(out_avals),
            in_names=tuple(all_in_names),
            out_names=tuple(out_names),
            lowering_input_output_aliases=(),
            sim_require_finite=True,
            sim_require_nnan=True,
            nc=nc,
        )
        return tuple(outs)

    devices = jax.devices()[:N_CORES]
    mesh = Mesh(np.asarray(devices), ("core",))
    sharded = jax.jit(
        shard_map(
            _body, mesh=mesh,
            in_specs=(PartitionSpec("core"),) * (n_params + n_outs),
            out_specs=(PartitionSpec("core"),) * n_outs,
            check_rep=False,
        ),
        donate_argnums=donate,
        keep_unused=True,
    )

    def run(in_maps):
        per_core = [[np.asarray(m[name]) for name in in_names] for m in in_maps]
        concat_in = [
            np.concatenate([per_core[c][i] for c in range(N_CORES)], axis=0)
            for i in range(n_params)
        ]
        concat_zeros = [
            np.zeros((N_CORES * s[0], *s[1:]), d) for (s, d) in out_shapes
        ]
        out_arrs = sharded(*concat_in, *concat_zeros)
        return [
            {
                name: np.asarray(out_arrs[i]).reshape(N_CORES, *out_shapes[i][0])[c]
                for i, name in enumerate(out_names)
            }
            for c in range(N_CORES)
        ]

    _CACHE[key] = run
    return run


# ------------------------------------------------------------------- host
def _prep_inputs(x, Wkv, bkv, Wq, bq, Wp, bp, buffer, sample_lengths):
    x = np.asarray(x, np.float32)
    Wkv = np.asarray(Wkv, np.float32)
    bkv = np.asarray(bkv, np.float32)
    Wq = np.asarray(Wq, np.float32)
    bq = np.asarray(bq, np.float32)
    Wp = np.asarray(Wp, np.float32)
    bp = np.asarray(bp, np.float32)
    buffer = np.asarray(buffer, np.float32)
    lengths = np.asarray(sample_lengths).astype(np.int64)

    scale = 1.0 / math.sqrt(DK)
    starts = np.concatenate([[0], np.cumsum(lengths)[:-1]]).astype(np.int64)
    t = np.arange(T)
    seg = np.searchsorted(starts, t, side="right") - 1
    j = t - starts[seg]

    wk = np.ascontiguousarray(Wkv[:KD, :].T)
    wv_aug = np.zeros((E, H, 65), np.float32)
    wv_aug[:, :, :64] = Wkv[KD:, :].T.reshape(E, H, DV)
    wv = np.ascontiguousarray(wv_aug.reshape(E, H * 65))
    wq = np.ascontiguousarray(Wq.T * scale)
    wp = np.ascontiguousarray(Wp.T)
    bk2 = np.ascontiguousarray(bkv[:KD].reshape(4, 128).T)
    bq2 = np.ascontiguousarray((bq * scale).reshape(4, 128).T)
    bv_aug = np.zeros((H, 65), np.float32)
    bv_aug[:, :64] = bkv[KD:].reshape(H, DV)
    bv_aug[:, 64] = 1.0
    bv_row = np.ascontiguousarray(bv_aug.reshape(1, H * 65))
    bp_row = np.ascontiguousarray(bp[None, :])
    ones_row = np.ones((1, 128), np.float32)

    kbufT = np.zeros((KD, 32), np.float32)
    kbufT[:, :HALO] = buffer[:, :KD].T
    vbuf = np.zeros((32, H * 65), np.float32)
    vb = vbuf.reshape(32, H, 65)
    vb[:HALO, :, :64] = buffer[:, KD:].reshape(HALO, H, DV)
    vb[:HALO, :, 64] = 1.0

    xTp = np.zeros((E, T + HALO + 33), np.float32)
    xTp[:, HALO:HALO + T] = x.T

    in_maps = []
    for c in range(N_CORES):
        t0 = c * SHARD
        xT_c = np.ascontiguousarray(xTp[:, t0:t0 + NTOK])
        mask = np.full((NB, 128, CTXF), NEG, np.float32)
        for bblk in range(NB):
            i = np.arange(TB)
            tt = t0 + bblk * TB + i
            st = starts[seg[tt]]
            jj = j[tt]
            for r in range(2):
                p = np.arange(128)[:, None]
                g = t0 - HALO + bblk * TB + r * 128 + p
                valid = (
                    (g >= tt[None, :] - HALO) & (g <= tt[None, :])
                    & (g >= st[None, :]) & (g >= 0) & (g < T)
                )
                mask[bblk, :, r * 256:(r + 1) * 256] = np.where(valid, 0.0, NEG)
            p = np.arange(32)[:, None]
            g = t0 - HALO + bblk * TB + 256 + p
            valid = (
                (g >= tt[None, :] - HALO) & (g <= tt[None, :])
                & (g >= st[None, :]) & (g >= 0) & (g < T)
            )
            mask[bblk, 0:32, 512:768] = np.where(valid, 0.0, NEG)
            pb = np.arange(32)[:, None]
            validb = (pb >= jj[None, :]) & (pb <= HALO - 1)
            mask[bblk, 0:32, 768:1024] = np.where(validb, 0.0, NEG)
        in_maps.append({
            "xT": xT_c, "wk": wk, "wv": wv, "wq": wq, "wp": wp,
            "bk": bk2, "bq": bq2, "bv": bv_row, "bp": bp_row,
            "ones": ones_row, "kbufT": kbufT, "vbuf": vbuf,
            "mask": np.ascontiguousarray(mask),
        })
    return in_maps, seg, j


def kernel(x, Wkv, bkv, Wq, bq, Wp, bp, buffer, sample_lengths):
    in_maps, seg, j = _prep_inputs(
        x, Wkv, bkv, Wq, bq, Wp, bp, buffer, sample_lengths
    )
    with_bias = bool(
        np.any(np.asarray(bkv)) or np.any(np.asarray(bq)) or np.any(np.asarray(bp))
    )
    run = _get_runner(with_bias)
    results = run(in_maps)
    out_full = np.concatenate([results[c]["yout"] for c in range(N_CORES)], axis=0)
    y = np.zeros((B, MAXL, E), np.float32)
    ok = j < MAXL
    y[seg[ok], j[ok]] = out_full[ok]
    return y



# revision 11
# speedup vs baseline: 1.2595x; 1.2595x over previous
"""Trainium2 Bass kernel for nn_DistiledMultiheadAttention_76476187673064.

Sliding-window (W=32) single-query attention over ragged sequences with a
learned pre-context buffer, plus input/output projections.

Strategy (8 NeuronCores, data-parallel over flat tokens):
  - Each core owns 512 tokens; kv for a 31-token halo is recomputed locally
    (plus one masked pad column), so no collectives are needed.
  - All matmul operands are bf16 (host-cast, fp32 PSUM accumulation):
    halves HBM traffic and LDWEIGHTS time, and removes the f32r
    narrow-output rate penalty.
  - Host passes pre-rearranged weights/activations so every DMA is a
    straight [128, N] partition-major copy (one big descriptor per
    partition) and every matmul's contraction lands on SBUF partitions:
      * K/Q projections feature-major (kT, qT: [feat, tok])
      * V projection token-major, augmented with a ones column per head
        (so PV emits per-head softmax sums for free)
      * QK logits ctx-major [ctx, tok]; the tail+buffer columns of 4 heads
        are packed into one full 128-partition PSUM tile (1/4 the mask/exp
        work); band+segment+buffer masking via a host-precomputed additive
        mask; exp without max-subtraction (logits are bounded)
  - ScalarE runs ONLY Exp + Copy (both in one activation table -> a single
    ACT_TABLE_LOAD for the whole kernel; the baseline's Exp<->Reciprocal
    alternation cost 32 table loads = 41us).  All 32 softmax-sum rows are
    gathered into one [32, 256] tile and reciprocated by a single DVE
    InstReciprocal; normalization is applied as rank-1 broadcast matmuls
    (ones (x) recip row) multiplied into the attention output.
"""
import math
import sys

sys.path.insert(0, "/opt/trn_rl_repo")

import numpy as np

# ---------------------------------------------------------------- constants
T = 4096
E = 1024
KD = 512          # key dim
H = 16            # heads
W = 32            # window
DK = KD // H      # 32
DV = E // H       # 64
B = 8
MAXL = 768
N_CORES = 8
SHARD = T // N_CORES          # 512 tokens per core
HALO = W - 1                  # 31
NTOK = SHARD + HALO + 1       # 544 token columns incl. halo + 1 pad
TB = 256                      # attention token block
NB = SHARD // TB              # 2 blocks per core
NEG = -30000.0

_CACHE = {}


# ------------------------------------------------------------- tile patches
def _apply_tile_patches():
    """This container's walrus only supports ONE sync-wait per instruction;
    redistribute extra Tile-assigned waits onto single-wait InstNoOp carriers."""
    import concourse.mybir as mybir
    import concourse.tile as tile
    from concourse.vector_clock import ScopedClock

    if getattr(tile.TileContext, "_wait_split_patched", False):
        return
    orig_commit = tile.TileContext._commit_and_lower

    def commit_split(self, inst, original_block, old_bb_map, bb_to_exit_bb):
        si = getattr(inst, "sync_info", None)
        if si is not None and si.on_wait and len(si.on_wait) > 1:
            engine = inst.engine
            if engine is not None and engine != mybir.EngineType.Unassigned:
                waits = list(si.on_wait)
                si.on_wait = waits[-1:]
                for w in waits[:-1]:
                    noop = mybir.InstNoOp(
                        name=self.nc.get_next_instruction_name(),
                        sync_info=mybir.SyncInfo(on_wait=[w], on_update=[]),
                        bass_nofuse=True,
                        engine=engine,
                        text_hint="wait_split",
                    )
                    orig_commit(self, noop, original_block, old_bb_map, bb_to_exit_bb)
        return orig_commit(self, inst, original_block, old_bb_map, bb_to_exit_bb)

    def drain_and_barrier(self, tick_clock, wait_clock):
        drain_inst = self.nc.sync.drain()
        wait_clock.add_sem_waits(
            drain_inst.ins, ScopedClock({None: tick_clock.global_clock})
        )
        si = drain_inst.ins.sync_info
        if si is not None and si.on_wait and len(si.on_wait) > 1:
            waits = list(si.on_wait)
            si.on_wait = waits[:1]
            for w in waits[1:]:
                nop = self.nc.sync.nop(nofuse=True)
                nsi = nop.ins.sync_info
                if nsi is None:
                    nop.ins.sync_info = mybir.SyncInfo(on_wait=[w], on_update=[])
                else:
                    nsi.on_wait = list(nsi.on_wait or []) + [w]
        self.nc.all_engine_barrier()
        assert self.sems is not None
        popped = self.nc._tile_sem_poison_stack.pop()
        assert popped is self._sem_poison
        self.nc.clear_and_free_semaphores(list(self.sems.allocated().values()))
        self.nc.all_engine_barrier()

    tile.TileContext._commit_and_lower = commit_split
    tile.TileContext._drain_and_barrier = drain_and_barrier
    tile.TileContext._wait_split_patched = True


# ------------------------------------------------------------- device build
def _build_nc(with_bias=True):
    import concourse.bass as bass
    import concourse.mybir as mybir
    import concourse.tile as tile

    _apply_tile_patches()
    f32 = mybir.dt.float32
    f32r = mybir.dt.float32r
    bf16 = mybir.dt.bfloat16
    ADD = mybir.AluOpType.add
    MUL = mybir.AluOpType.mult
    EXP = mybir.ActivationFunctionType.Exp

    nc = bass.Bass()
    d_xT = nc.dram_tensor("xT", [128, 8, NTOK], bf16, kind="ExternalInput")
    d_wk = nc.dram_tensor("wk", [128, 4, 8, 128], bf16, kind="ExternalInput")
    d_wq = nc.dram_tensor("wq", [128, 4, 8, 128], bf16, kind="ExternalInput")
    d_wv = nc.dram_tensor("wv", [128, 4, 8, 260], bf16, kind="ExternalInput")
    d_wp = nc.dram_tensor("wp", [128, 2, 8, 512], bf16, kind="ExternalInput")
    d_kbufT = nc.dram_tensor("kbufT", [128, 4, 32], bf16, kind="ExternalInput")
    d_vbuf4 = nc.dram_tensor("vbuf4", [128, 1040], bf16, kind="ExternalInput")
    d_ones = nc.dram_tensor("ones", [1, 128], f32r, kind="ExternalInput")
    d_mask = nc.dram_tensor("mask", [128, NB, 1024], f32, kind="ExternalInput")
    d_bk = nc.dram_tensor("bk", [128, 4], f32, kind="ExternalInput")
    d_bq = nc.dram_tensor("bq", [128, 4], f32, kind="ExternalInput")
    d_bv = nc.dram_tensor("bv", [1, H * 65], f32r, kind="ExternalInput")
    d_bp = nc.dram_tensor("bp", [1, E], f32r, kind="ExternalInput")
    d_y = nc.dram_tensor("yout", [SHARD, E], f32, kind="ExternalOutput")

    with tile.TileContext(nc) as tc, nc.allow_low_precision(
        reason="bf16 matmul operands; fp32 PSUM accumulation throughout"
    ):
        with (
            tc.tile_pool(name="x", bufs=1) as x_pool,
            tc.tile_pool(name="wgt", bufs=1) as w_pool,
            tc.tile_pool(name="const", bufs=1) as const_pool,
            tc.tile_pool(name="kqv", bufs=1) as kqv_pool,
            tc.tile_pool(name="exp", bufs=12) as exp_pool,
            tc.tile_pool(name="rrow", bufs=4) as r_pool,
            tc.tile_pool(name="srow", bufs=4) as s_pool,
            tc.tile_pool(name="out", bufs=3) as out_pool,
        ):
            # ---- x first (feature-major, all tokens incl. halo+pad)
            xT = x_pool.tile([128, 8, NTOK], bf16)
            nc.sync.dma_start(xT[:], d_xT[:])

            # ---- weights as whole tiles, in consumption order
            wk_sb = w_pool.tile([128, 4, 8, 128], bf16)
            nc.sync.dma_start(wk_sb[:], d_wk[:])
            wq_sb = w_pool.tile([128, 4, 8, 128], bf16)
            nc.sync.dma_start(wq_sb[:], d_wq[:])
            wv_sb = w_pool.tile([128, 4, 8, 260], bf16)
            nc.sync.dma_start(wv_sb[:], d_wv[:])

            # ---- attention constants
            kbuf_sb = const_pool.tile([128, 4, 32], bf16)
            nc.sync.dma_start(kbuf_sb[:], d_kbufT[:])
            vbuf_sb = const_pool.tile([128, 1040], bf16)
            nc.sync.dma_start(vbuf_sb[:], d_vbuf4[:])
            ones = const_pool.tile([1, 128], f32r)
            nc.sync.dma_start(ones[:], d_ones[:])
            mask_sb = const_pool.tile([128, NB, 1024], f32)
            nc.sync.dma_start(mask_sb[:], d_mask[:])
            if with_bias:
                bk_sb = const_pool.tile([128, 4], f32)
                nc.sync.dma_start(bk_sb[:], d_bk[:])
                bq_sb = const_pool.tile([128, 4], f32)
                nc.sync.dma_start(bq_sb[:], d_bq[:])
                bv_sb = const_pool.tile([1, H * 65], f32r)
                nc.sync.dma_start(bv_sb[:], d_bv[:])
                bp_sb = const_pool.tile([1, E], f32r)
                nc.sync.dma_start(bp_sb[:], d_bp[:])

            # ---- output-projection weights last (consumed last)
            wp_sb = w_pool.tile([128, 2, 8, 512], bf16)
            nc.sync.dma_start(wp_sb[:], d_wp[:])

            # ---- persistent activations
            kT = kqv_pool.tile([128, 4, NTOK], bf16)    # K feature-major
            qT = kqv_pool.tile([128, 4, SHARD], bf16)   # Q feature-major (scaled)
            vA = kqv_pool.tile([128, 5, H * 65], bf16)  # V token-major + ones col
            vTail = kqv_pool.tile([128, NB, H * 65], bf16)  # tail-ctx V, 4x replicated
            oT = kqv_pool.tile([128, 8, SHARD], bf16)   # attention out feature-major
            s_all = kqv_pool.tile([32, TB], f32)        # softmax sums, row per (b,h)
            r_all = kqv_pool.tile([32, TB], f32r)       # their reciprocals

            with tc.tile_pool(name="pp1", bufs=2, space="PSUM") as pp1:
                # K projection: kT[:, m, :] = wk[:, mchunk].T @ xT (+ bk)
                for m in range(4):
                    pa = pp1.tile([128, 512], f32, tag="pa")
                    pb = pp1.tile([128, 32], f32, tag="pb")
                    for e in range(8):
                        nc.tensor.matmul(
                            pa[:], wk_sb[:, m, e, :], xT[:, e, 0:512],
                            start=(e == 0), stop=(e == 7),
                        )
                        nc.tensor.matmul(
                            pb[:], wk_sb[:, m, e, :], xT[:, e, 512:NTOK],
                            start=(e == 0), stop=(e == 7),
                        )
                    if with_bias:
                        nc.scalar.add(kT[:, m, 0:512], pa[:], bk_sb[:, m:m + 1])
                        nc.scalar.add(kT[:, m, 512:NTOK], pb[:], bk_sb[:, m:m + 1])
                    else:
                        nc.scalar.copy(kT[:, m, 0:512], pa[:])
                        nc.scalar.copy(kT[:, m, 512:NTOK], pb[:])

                # Q projection (tokens only, no halo)
                for m in range(4):
                    pa = pp1.tile([128, 512], f32, tag="pa")
                    for e in range(8):
                        nc.tensor.matmul(
                            pa[:], wq_sb[:, m, e, :], xT[:, e, HALO:HALO + SHARD],
                            start=(e == 0), stop=(e == 7),
                        )
                    if with_bias:
                        nc.scalar.add(qT[:, m, :], pa[:], bq_sb[:, m:m + 1])
                    else:
                        nc.scalar.copy(qT[:, m, :], pa[:])

                # V projection token-major (wv pre-augmented with zero ones-cols):
                # vA[tok, h*65:h*65+65] = [x @ Wv_h.T + bv_h | 1]
                tok_sizes = [128, 128, 128, 128, 32]
                for f in range(4):
                    for i in range(5):
                        mt = tok_sizes[i]
                        pa = pp1.tile([128, 260], f32, tag="pv")
                        for e in range(8):
                            nc.tensor.matmul(
                                pa[0:mt, :],
                                xT[:, e, i * 128:i * 128 + mt],
                                wv_sb[:, f, e, :],
                                start=(e == 0), stop=(e == 7 and not with_bias),
                            )
                        if with_bias:
                            nc.tensor.matmul(
                                pa[0:mt, :], ones[0:1, 0:mt],
                                bv_sb[0:1, f * 260:(f + 1) * 260],
                                start=False, stop=True,
                            )
                        nc.vector.tensor_copy(
                            vA[0:mt, i, f * 260:(f + 1) * 260], pa[0:mt, :]
                        )
                        if not with_bias:
                            # ones columns via strided add (psum zeros there)
                            ov_view = vA[0:mt, i, f * 260:(f + 1) * 260].rearrange(
                                "p (h c) -> p h c", c=65
                            )[:, :, 64:65]
                            nc.gpsimd.tensor_scalar_add(ov_view, ov_view, 1.0)

            # replicate the per-block tail-ctx V rows across all four
            # 32-partition groups so packed-tail PV matmuls line up
            for b in range(NB):
                for r in range(4):
                    nc.sync.dma_start(
                        vTail[r * 32:(r + 1) * 32, b, :], vA[0:32, 2 * b + 2, :]
                    )

            # ---- attention
            with (
                tc.tile_pool(name="plgM", bufs=4, space="PSUM") as plgM,
                tc.tile_pool(name="plgT", bufs=2, space="PSUM") as plgT,
                tc.tile_pool(name="pov", bufs=2, space="PSUM") as pov,
            ):
                prev = None
                for it in range(NB * 4 + 1):
                    if it < NB * 4:
                        b, g = divmod(it, 4)
                        base = b * TB
                        lgMs = []
                        for hh in range(4):
                            ro = hh * 32
                            qh = qT[ro:ro + 32, g, base:base + TB]
                            lg = plgM.tile([128, 512], f32)
                            nc.tensor.matmul(
                                lg[:, 0:256], kT[ro:ro + 32, g, base:base + 128],
                                qh, start=True, stop=True, tile_position=(ro, 0),
                            )
                            nc.tensor.matmul(
                                lg[:, 256:512],
                                kT[ro:ro + 32, g, base + 128:base + 256],
                                qh, start=True, stop=True, tile_position=(ro, 0),
                            )
                            lgMs.append(lg)
                        lgT = plgT.tile([128, 512], f32)
                        for hh in range(4):
                            ro = hh * 32
                            qh = qT[ro:ro + 32, g, base:base + TB]
                            nc.tensor.matmul(
                                lgT[ro:ro + 32, 0:256],
                                kT[ro:ro + 32, g, base + 256:base + 288],
                                qh, start=True, stop=True, tile_position=(ro, ro),
                            )
                            nc.tensor.matmul(
                                lgT[ro:ro + 32, 256:512],
                                kbuf_sb[ro:ro + 32, g, :],
                                qh, start=True, stop=True, tile_position=(ro, ro),
                            )
                        exMs = []
                        for hh in range(4):
                            nc.vector.tensor_tensor(
                                lgMs[hh][:], lgMs[hh][:], mask_sb[:, b, 0:512], ADD
                            )
                            ex = exp_pool.tile([128, 512], bf16)
                            nc.scalar.activation(ex[:], lgMs[hh][:], EXP)
                            exMs.append(ex)
                        nc.vector.tensor_tensor(
                            lgT[:], lgT[:], mask_sb[:, b, 512:1024], ADD
                        )
                        exT = exp_pool.tile([128, 512], bf16)
                        nc.scalar.activation(exT[:], lgT[:], EXP)
                        cur = (b, g, base, exMs, exT)
                    else:
                        cur = None
                    if prev is not None:
                        b, g, base, exMs, exT = prev
                        for hh in range(4):
                            h = g * 4 + hh
                            ro = hh * 32
                            hc = h * 65
                            ov = pov.tile([128, 256], f32)
                            nc.tensor.matmul(
                                ov[0:65, :], vA[:, 2 * b, hc:hc + 65],
                                exMs[hh][:, 0:256], start=True, stop=False,
                            )
                            nc.tensor.matmul(
                                ov[0:65, :], vA[:, 2 * b + 1, hc:hc + 65],
                                exMs[hh][:, 256:512], start=False, stop=False,
                            )
                            nc.tensor.matmul(
                                ov[0:65, :], vTail[ro:ro + 32, b, hc:hc + 65],
                                exT[ro:ro + 32, 0:256], start=False, stop=False,
                                tile_position=(ro, 0),
                            )
                            nc.tensor.matmul(
                                ov[0:65, :], vbuf_sb[ro:ro + 32, hc:hc + 65],
                                exT[ro:ro + 32, 256:512], start=False, stop=True,
                                tile_position=(ro, 0),
                            )
                            od = oT[(h % 2) * 64:(h % 2) * 64 + 64, h // 2,
                                    base:base + TB]
                            nc.scalar.copy(od, ov[0:64, :])
                            idx = b * H + h
                            # engine copies need 32-aligned partition offsets,
                            # and DMA cannot read PSUM: stage the sum row at
                            # partition 0, then DMA it to its s_all row
                            sst = s_pool.tile([1, TB], f32)
                            nc.scalar.copy(sst[:], ov[64:65, :])
                            nc.sync.dma_start(s_all[idx:idx + 1, :], sst[:])
                    prev = cur

            # ---- softmax normalization: one batched reciprocal, then
            # rank-1 broadcast (ones (x) r) multiplied into oT
            nc.vector.reciprocal(r_all[:], s_all[:])
            with tc.tile_pool(name="prb", bufs=2, space="PSUM") as prb:
                for b in range(NB):
                    for h in range(H):
                        idx = b * H + h
                        r0 = r_pool.tile([1, TB], f32r)
                        nc.sync.dma_start(r0[:], r_all[idx:idx + 1, :])
                        rb = prb.tile([64, TB], f32)
                        nc.tensor.matmul(
                            rb[:], ones[0:1, 0:64], r0[:], start=True, stop=True,
                        )
                        sl = oT[(h % 2) * 64:(h % 2) * 64 + 64, h // 2,
                                b * TB:(b + 1) * TB]
                        nc.vector.tensor_tensor(sl, sl, rb[:], MUL)

            # ---- output projection: y[tok, :] = oT.T @ wp (+ bp)
            with tc.tile_pool(name="pp3", bufs=2, space="PSUM") as pp3:
                for f in range(2):
                    for m in range(4):
                        pa = pp3.tile([128, 512], f32)
                        for c in range(8):
                            nc.tensor.matmul(
                                pa[:], oT[:, c, m * 128:(m + 1) * 128],
                                wp_sb[:, f, c, :], start=(c == 0),
                                stop=(c == 7 and not with_bias),
                            )
                        if with_bias:
                            nc.tensor.matmul(
                                pa[:], ones[0:1, 0:128],
                                bp_sb[0:1, f * 512:(f + 1) * 512],
                                start=False, stop=True,
                            )
                        ot = out_pool.tile([128, 512], f32)
                        nc.vector.tensor_copy(ot[:], pa[:])
                        nc.sync.dma_start(
                            d_y[m * 128:(m + 1) * 128, f * 512:(f + 1) * 512], ot[:]
                        )
    return nc


def _get_runner(with_bias=True):
    key = ("runner", with_bias)
    if key in _CACHE:
        return _CACHE[key]
    import jax
    import concourse.mybir as mybir
    from concourse import bass2jax
    from jax.sharding import Mesh, PartitionSpec
    from jax.experimental.shard_map import shard_map

    nc = _build_nc(with_bias)
    bass2jax.install_neuronx_cc_hook()
    partition_name = nc.partition_id_tensor.name if nc.partition_id_tensor else None
    in_names, out_names, out_avals, out_shapes = [], [], [], []
    for alloc in nc.m.functions[0].allocations:
        if not isinstance(alloc, mybir.MemoryLocationSet):
            continue
        name = alloc.memorylocations[0].name
        if alloc.kind == "ExternalInput":
            if name != partition_name:
                in_names.append(name)
        elif alloc.kind == "ExternalOutput":
            shape = tuple(alloc.tensor_shape)
            dtype = mybir.dt.np(alloc.dtype)
            out_names.append(name)
            out_avals.append(jax.core.ShapedArray(shape, dtype))
            out_shapes.append((shape, dtype))
    n_params = len(in_names)
    n_outs = len(out_avals)
    all_in_names = in_names + out_names + ([partition_name] if partition_name else [])
    donate = tuple(range(n_params, n_params + n_outs))

    def _body(*args):
        operands = list(args)
        if partition_name is not None:
            operands.append(bass2jax.partition_id_tensor())
        outs = bass2jax._bass_exec_p.bind(
            *operands,
            out_avals=tuple(out_avals),
            in_names=tuple(all_in_names),
            out_names=tuple(out_names),
            lowering_input_output_aliases=(),
            sim_require_finite=True,
            sim_require_nnan=True,
            nc=nc,
        )
        return tuple(outs)

    devices = jax.devices()[:N_CORES]
    mesh = Mesh(np.asarray(devices), ("core",))
    sharded = jax.jit(
        shard_map(
            _body, mesh=mesh,
            in_specs=(PartitionSpec("core"),) * (n_params + n_outs),
            out_specs=(PartitionSpec("core"),) * n_outs,
            check_rep=False,
        ),
        donate_argnums=donate,
        keep_unused=True,
    )

    def run(in_maps):
        per_core = [[np.asarray(m[name]) for name in in_names] for m in in_maps]
        concat_in = [
            np.concatenate([per_core[c][i] for c in range(N_CORES)], axis=0)
            for i in range(n_params)
        ]
        concat_zeros = [
            np.zeros((N_CORES * s[0], *s[1:]), d) for (s, d) in out_shapes
        ]
        out_arrs = sharded(*concat_in, *concat_zeros)
        return [
            {
                name: np.asarray(out_arrs[i]).reshape(N_CORES, *out_shapes[i][0])[c]
                for i, name in enumerate(out_names)
            }
            for c in range(N_CORES)
        ]

    _CACHE[key] = run
    return run


# ------------------------------------------------------------------- host
def _prep_inputs(x, Wkv, bkv, Wq, bq, Wp, bp, buffer, sample_lengths):
    import ml_dtypes

    bfl = ml_dtypes.bfloat16
    x = np.asarray(x, np.float32)
    Wkv = np.asarray(Wkv, np.float32)
    bkv = np.asarray(bkv, np.float32)
    Wq = np.asarray(Wq, np.float32)
    bq = np.asarray(bq, np.float32)
    Wp = np.asarray(Wp, np.float32)
    bp = np.asarray(bp, np.float32)
    buffer = np.asarray(buffer, np.float32)
    lengths = np.asarray(sample_lengths).astype(np.int64)

    scale = 1.0 / math.sqrt(DK)
    starts = np.concatenate([[0], np.cumsum(lengths)[:-1]]).astype(np.int64)
    t = np.arange(T)
    seg = np.searchsorted(starts, t, side="right") - 1
    j = t - starts[seg]

    # weights pre-rearranged into exact SBUF layouts ([p, ...] partition-major)
    wkT = np.ascontiguousarray(Wkv[:KD, :].T)                       # [E, KD]
    wk_h = wkT.reshape(8, 128, 4, 128).transpose(1, 2, 0, 3).astype(bfl)
    wqT = np.ascontiguousarray(Wq.T * scale)                        # [E, KD]
    wq_h = wqT.reshape(8, 128, 4, 128).transpose(1, 2, 0, 3).astype(bfl)
    wv_aug = np.zeros((E, H, 65), np.float32)
    wv_aug[:, :, :64] = Wkv[KD:, :].T.reshape(E, H, DV)
    wv_h = (
        wv_aug.reshape(E, H * 65).reshape(8, 128, 4, 260)
        .transpose(1, 2, 0, 3).astype(bfl)
    )
    wpT = np.ascontiguousarray(Wp.T)                                # [E, E]
    wp_h = wpT.reshape(8, 128, 2, 512).transpose(1, 2, 0, 3).astype(bfl)

    bk2 = np.ascontiguousarray(bkv[:KD].reshape(4, 128).T)
    bq2 = np.ascontiguousarray((bq * scale).reshape(4, 128).T)
    bv_aug = np.zeros((H, 65), np.float32)
    bv_aug[:, :64] = bkv[KD:].reshape(H, DV)
    bv_aug[:, 64] = 1.0
    bv_row = np.ascontiguousarray(bv_aug.reshape(1, H * 65))
    bp_row = np.ascontiguousarray(bp[None, :])
    ones_row = np.ones((1, 128), np.float32)

    kbufT = np.zeros((KD, 32), np.float32)
    kbufT[:, :HALO] = buffer[:, :KD].T
    kbuf_h = kbufT.reshape(4, 128, 32).transpose(1, 0, 2).astype(bfl)
    vbuf = np.zeros((32, H * 65), np.float32)
    vb = vbuf.reshape(32, H, 65)
    vb[:HALO, :, :64] = buffer[:, KD:].reshape(HALO, H, DV)
    vb[:HALO, :, 64] = 1.0
    vbuf4_h = np.tile(vbuf, (4, 1)).astype(bfl)

    xTp = np.zeros((E, T + HALO + 33), np.float32)
    xTp[:, HALO:HALO + T] = x.T

    in_maps = []
    for c in range(N_CORES):
        t0 = c * SHARD
        xT_c = np.ascontiguousarray(
            xTp[:, t0:t0 + NTOK].reshape(8, 128, NTOK).transpose(1, 0, 2)
        ).astype(bfl)
        mask = np.full((128, NB, 1024), NEG, np.float32)
        for bblk in range(NB):
            i = np.arange(TB)
            tt = t0 + bblk * TB + i
            st = starts[seg[tt]]
            jj = j[tt]
            for r in range(2):
                p = np.arange(128)[:, None]
                g = t0 - HALO + bblk * TB + r * 128 + p
                valid = (
                    (g >= tt[None, :] - HALO) & (g <= tt[None, :])
                    & (g >= st[None, :]) & (g >= 0) & (g < T)
                )
                mask[:, bblk, r * 256:(r + 1) * 256] = np.where(valid, 0.0, NEG)
            p32 = np.arange(32)[:, None]
            g = t0 - HALO + bblk * TB + 256 + p32
            valid = (
                (g >= tt[None, :] - HALO) & (g <= tt[None, :])
                & (g >= st[None, :]) & (g >= 0) & (g < T)
            )
            tailm = np.where(valid, 0.0, NEG)
            pb = np.arange(32)[:, None]
            validb = (pb >= jj[None, :]) & (pb <= HALO - 1)
            bufm = np.where(validb, 0.0, NEG)
            for rr in range(4):
                mask[rr * 32:(rr + 1) * 32, bblk, 512:768] = tailm
                mask[rr * 32:(rr + 1) * 32, bblk, 768:1024] = bufm
        in_maps.append({
            "xT": xT_c, "wk": wk_h, "wq": wq_h, "wv": wv_h, "wp": wp_h,
            "kbufT": kbuf_h, "vbuf4": vbuf4_h, "ones": ones_row,
            "mask": np.ascontiguousarray(mask),
            "bk": bk2, "bq": bq2, "bv": bv_row, "bp": bp_row,
        })
    return in_maps, seg, j


def kernel(x, Wkv, bkv, Wq, bq, Wp, bp, buffer, sample_lengths):
    in_maps, seg, j = _prep_inputs(
        x, Wkv, bkv, Wq, bq, Wp, bp, buffer, sample_lengths
    )
    with_bias = bool(
        np.any(np.asarray(bkv)) or np.any(np.asarray(bq)) or np.any(np.asarray(bp))
    )
    run = _get_runner(with_bias)
    results = run(in_maps)
    out_full = np.concatenate([results[c]["yout"] for c in range(N_CORES)], axis=0)
    y = np.zeros((B, MAXL, E), np.float32)
    ok = j < MAXL
    y[seg[ok], j[ok]] = out_full[ok]
    return y


# revision 18
# speedup vs baseline: 1.3397x; 1.0636x over previous
"""Trainium2 Bass kernel for nn_DistiledMultiheadAttention_76476187673064.

Sliding-window (W=32) single-query attention over ragged sequences with a
learned pre-context buffer, plus input/output projections.

Strategy (8 NeuronCores, data-parallel over flat tokens):
  - Each core owns 512 tokens; kv for a 31-token halo is recomputed locally
    (plus one masked pad column), so no collectives are needed.
  - All matmul operands are bf16 (host-cast, fp32 PSUM accumulation):
    halves HBM traffic and LDWEIGHTS time, and removes the f32r
    narrow-output rate penalty.
  - Host passes pre-rearranged weights/activations so every DMA is a
    straight [128, N] partition-major copy (one big descriptor per
    partition) and every matmul's contraction lands on SBUF partitions:
      * K/Q projections feature-major (kT, qT: [feat, tok])
      * V projection token-major, augmented with a ones column per head
        (so PV emits per-head softmax sums for free)
      * QK logits ctx-major [ctx, tok]; the tail+buffer columns of 4 heads
        are packed into one full 128-partition PSUM tile (1/4 the mask/exp
        work); band+segment+buffer masking via a host-precomputed additive
        mask; exp without max-subtraction (logits are bounded)
  - ScalarE runs ONLY Exp + Copy (both in one activation table -> a single
    ACT_TABLE_LOAD for the whole kernel; the baseline's Exp<->Reciprocal
    alternation cost 32 table loads = 41us).  All 32 softmax-sum rows are
    gathered into one [32, 256] tile and reciprocated by a single DVE
    InstReciprocal; normalization is applied as rank-1 broadcast matmuls
    (ones (x) recip row) multiplied into the attention output.
"""
import math
import sys

sys.path.insert(0, "/opt/trn_rl_repo")

import numpy as np

# ---------------------------------------------------------------- constants
T = 4096
E = 1024
KD = 512          # key dim
H = 16            # heads
W = 32            # window
DK = KD // H      # 32
DV = E // H       # 64
B = 8
MAXL = 768
N_CORES = 8
SHARD = T // N_CORES          # 512 tokens per core
HALO = W - 1                  # 31
NTOK = SHARD + HALO + 1       # 544 token columns incl. halo + 1 pad
TB = 256                      # attention token block
NB = SHARD // TB              # 2 blocks per core
NEG = -30000.0

_CACHE = {}


# ------------------------------------------------------------- tile patches
def _apply_tile_patches():
    """This container's walrus only supports ONE sync-wait per instruction;
    redistribute extra Tile-assigned waits onto single-wait InstNoOp carriers."""
    import concourse.mybir as mybir
    import concourse.tile as tile
    from concourse.vector_clock import ScopedClock

    if getattr(tile.TileContext, "_wait_split_patched", False):
        return
    orig_commit = tile.TileContext._commit_and_lower

    def commit_split(self, inst, original_block, old_bb_map, bb_to_exit_bb):
        si = getattr(inst, "sync_info", None)
        if si is not None and si.on_wait and len(si.on_wait) > 1:
            engine = inst.engine
            if engine is not None and engine != mybir.EngineType.Unassigned:
                waits = list(si.on_wait)
                si.on_wait = waits[-1:]
                for w in waits[:-1]:
                    noop = mybir.InstNoOp(
                        name=self.nc.get_next_instruction_name(),
                        sync_info=mybir.SyncInfo(on_wait=[w], on_update=[]),
                        bass_nofuse=True,
                        engine=engine,
                        text_hint="wait_split",
                    )
                    orig_commit(self, noop, original_block, old_bb_map, bb_to_exit_bb)
        return orig_commit(self, inst, original_block, old_bb_map, bb_to_exit_bb)

    def drain_and_barrier(self, tick_clock, wait_clock):
        drain_inst = self.nc.sync.drain()
        wait_clock.add_sem_waits(
            drain_inst.ins, ScopedClock({None: tick_clock.global_clock})
        )
        si = drain_inst.ins.sync_info
        if si is not None and si.on_wait and len(si.on_wait) > 1:
            waits = list(si.on_wait)
            si.on_wait = waits[:1]
            for w in waits[1:]:
                nop = self.nc.sync.nop(nofuse=True)
                nsi = nop.ins.sync_info
                if nsi is None:
                    nop.ins.sync_info = mybir.SyncInfo(on_wait=[w], on_update=[])
                else:
                    nsi.on_wait = list(nsi.on_wait or []) + [w]
        self.nc.all_engine_barrier()
        assert self.sems is not None
        popped = self.nc._tile_sem_poison_stack.pop()
        assert popped is self._sem_poison
        self.nc.clear_and_free_semaphores(list(self.sems.allocated().values()))
        self.nc.all_engine_barrier()

    tile.TileContext._commit_and_lower = commit_split
    tile.TileContext._drain_and_barrier = drain_and_barrier
    tile.TileContext._wait_split_patched = True


# ------------------------------------------------------------- device build
def _build_nc(with_bias=True):
    import concourse.bass as bass
    import concourse.mybir as mybir
    import concourse.tile as tile

    _apply_tile_patches()
    f32 = mybir.dt.float32
    f32r = mybir.dt.float32r
    bf16 = mybir.dt.bfloat16
    ADD = mybir.AluOpType.add
    MUL = mybir.AluOpType.mult
    EXP = mybir.ActivationFunctionType.Exp

    nc = bass.Bass()
    d_xT = nc.dram_tensor("xT", [128, 8, NTOK], bf16, kind="ExternalInput")
    d_wk = nc.dram_tensor("wk", [128, 4, 8, 128], bf16, kind="ExternalInput")
    d_wq = nc.dram_tensor("wq", [128, 4, 8, 128], bf16, kind="ExternalInput")
    d_wv = nc.dram_tensor("wv", [128, 4, 8, 260], bf16, kind="ExternalInput")
    d_wp = nc.dram_tensor("wp", [128, 2, 8, 512], bf16, kind="ExternalInput")
    d_kbufT = nc.dram_tensor("kbufT", [128, 4, 32], bf16, kind="ExternalInput")
    d_vbuf4 = nc.dram_tensor("vbuf4", [128, 1040], bf16, kind="ExternalInput")
    d_ones = nc.dram_tensor("ones", [1, 128], f32r, kind="ExternalInput")
    d_mask = nc.dram_tensor("mask", [128, NB, 1024], f32, kind="ExternalInput")
    d_bk = nc.dram_tensor("bk", [128, 4], f32, kind="ExternalInput")
    d_bq = nc.dram_tensor("bq", [128, 4], f32, kind="ExternalInput")
    d_bv = nc.dram_tensor("bv", [1, H * 65], f32r, kind="ExternalInput")
    d_bp = nc.dram_tensor("bp", [1, E], f32r, kind="ExternalInput")
    d_y = nc.dram_tensor("yout", [SHARD, E], f32, kind="ExternalOutput")
    d_rsc = nc.dram_tensor("rscratch", [32, TB], f32, kind="Internal")

    with tile.TileContext(nc) as tc, nc.allow_low_precision(
        reason="bf16 matmul operands; fp32 PSUM accumulation throughout"
    ):
        with (
            tc.tile_pool(name="x", bufs=1) as x_pool,
            tc.tile_pool(name="wgt", bufs=1) as w_pool,
            tc.tile_pool(name="const", bufs=1) as const_pool,
            tc.tile_pool(name="kqv", bufs=1) as kqv_pool,
            tc.tile_pool(name="exp", bufs=12) as exp_pool,
            tc.tile_pool(name="rrow", bufs=4) as r_pool,
            tc.tile_pool(name="srow", bufs=4) as s_pool,
            tc.tile_pool(name="out", bufs=3) as out_pool,
        ):
            # ---- x first (feature-major, all tokens incl. halo+pad)
            xT = x_pool.tile([128, 8, NTOK], bf16)
            nc.sync.dma_start(xT[:], d_xT[:])

            # ---- weights as whole tiles, in consumption order
            wk_sb = w_pool.tile([128, 4, 8, 128], bf16)
            nc.sync.dma_start(wk_sb[:], d_wk[:])
            wq_sb = w_pool.tile([128, 4, 8, 128], bf16)
            nc.sync.dma_start(wq_sb[:], d_wq[:])
            wv_sb = w_pool.tile([128, 4, 8, 260], bf16)
            nc.sync.dma_start(wv_sb[:], d_wv[:])

            # ---- attention constants
            kbuf_sb = const_pool.tile([128, 4, 32], bf16)
            nc.sync.dma_start(kbuf_sb[:], d_kbufT[:])
            vbuf_sb = const_pool.tile([128, 1040], bf16)
            nc.sync.dma_start(vbuf_sb[:], d_vbuf4[:])
            ones = const_pool.tile([1, 128], f32r)
            nc.sync.dma_start(ones[:], d_ones[:])
            mask_sb = const_pool.tile([128, NB, 1024], f32)
            nc.sync.dma_start(mask_sb[:], d_mask[:])
            if with_bias:
                bk_sb = const_pool.tile([128, 4], f32)
                nc.sync.dma_start(bk_sb[:], d_bk[:])
                bq_sb = const_pool.tile([128, 4], f32)
                nc.sync.dma_start(bq_sb[:], d_bq[:])
                bv_sb = const_pool.tile([1, H * 65], f32r)
                nc.sync.dma_start(bv_sb[:], d_bv[:])
                bp_sb = const_pool.tile([1, E], f32r)
                nc.sync.dma_start(bp_sb[:], d_bp[:])

            # ---- output-projection weights last (consumed last)
            wp_sb = w_pool.tile([128, 2, 8, 512], bf16)
            nc.sync.dma_start(wp_sb[:], d_wp[:])

            # ---- persistent activations
            kT = kqv_pool.tile([128, 4, NTOK], bf16)    # K feature-major
            qT = kqv_pool.tile([128, 4, SHARD], bf16)   # Q feature-major (scaled)
            vA = kqv_pool.tile([128, 5, H * 65], bf16)  # V token-major + ones col
            vTail = kqv_pool.tile([128, NB, H * 65], bf16)  # tail-ctx V, 4x replicated
            oT = kqv_pool.tile([128, 8, SHARD], bf16)   # attention out feature-major
            s_all = kqv_pool.tile([32, TB], f32)        # softmax sums, row per (b,h)
            r_all = kqv_pool.tile([32, TB], f32)        # their reciprocals
            r_flat = kqv_pool.tile([1, 32 * TB], f32)   # recips gathered on one row

            with tc.tile_pool(name="pp1", bufs=2, space="PSUM") as pp1:
                # K projection: kT[:, m, :] = wk[:, mchunk].T @ xT (+ bk)
                for m in range(4):
                    pa = pp1.tile([128, 512], f32, tag="pa")
                    pb = pp1.tile([128, 32], f32, tag="pb")
                    for e in range(8):
                        nc.tensor.matmul(
                            pa[:], wk_sb[:, m, e, :], xT[:, e, 0:512],
                            start=(e == 0), stop=(e == 7),
                        )
                        nc.tensor.matmul(
                            pb[:], wk_sb[:, m, e, :], xT[:, e, 512:NTOK],
                            start=(e == 0), stop=(e == 7),
                        )
                    if with_bias:
                        nc.scalar.add(kT[:, m, 0:512], pa[:], bk_sb[:, m:m + 1])
                        nc.scalar.add(kT[:, m, 512:NTOK], pb[:], bk_sb[:, m:m + 1])
                    else:
                        nc.scalar.copy(kT[:, m, 0:512], pa[:])
                        nc.scalar.copy(kT[:, m, 512:NTOK], pb[:])

                # Q projection (tokens only, no halo)
                for m in range(4):
                    pa = pp1.tile([128, 512], f32, tag="pa")
                    for e in range(8):
                        nc.tensor.matmul(
                            pa[:], wq_sb[:, m, e, :], xT[:, e, HALO:HALO + SHARD],
                            start=(e == 0), stop=(e == 7),
                        )
                    if with_bias:
                        nc.scalar.add(qT[:, m, :], pa[:], bq_sb[:, m:m + 1])
                    else:
                        nc.scalar.copy(qT[:, m, :], pa[:])

                # V projection token-major (wv pre-augmented with zero ones-cols):
                # vA[tok, h*65:h*65+65] = [x @ Wv_h.T + bv_h | 1]
                tok_sizes = [128, 128, 128, 128, 32]
                for f in range(4):
                    for i in range(5):
                        mt = tok_sizes[i]
                        pa = pp1.tile([128, 260], f32, tag="pv")
                        for e in range(8):
                            nc.tensor.matmul(
                                pa[0:mt, :],
                                xT[:, e, i * 128:i * 128 + mt],
                                wv_sb[:, f, e, :],
                                start=(e == 0), stop=(e == 7 and not with_bias),
                            )
                        if with_bias:
                            nc.tensor.matmul(
                                pa[0:mt, :], ones[0:1, 0:mt],
                                bv_sb[0:1, f * 260:(f + 1) * 260],
                                start=False, stop=True,
                            )
                        nc.vector.tensor_copy(
                            vA[0:mt, i, f * 260:(f + 1) * 260], pa[0:mt, :]
                        )
                        if not with_bias:
                            # ones columns via strided add (psum zeros there)
                            ov_view = vA[0:mt, i, f * 260:(f + 1) * 260].rearrange(
                                "p (h c) -> p h c", c=65
                            )[:, :, 64:65]
                            nc.gpsimd.tensor_scalar_add(ov_view, ov_view, 1.0)

            # replicate the per-block tail-ctx V rows across all four
            # 32-partition groups so packed-tail PV matmuls line up
            for b in range(NB):
                for r in range(4):
                    nc.sync.dma_start(
                        vTail[r * 32:(r + 1) * 32, b, :], vA[0:32, 2 * b + 2, :]
                    )

            # ---- attention
            with (
                tc.tile_pool(name="plgM", bufs=4, space="PSUM") as plgM,
                tc.tile_pool(name="plgT", bufs=2, space="PSUM") as plgT,
                tc.tile_pool(name="pov", bufs=2, space="PSUM") as pov,
            ):
                prev = None
                for it in range(NB * 4 + 1):
                    if it < NB * 4:
                        b, g = divmod(it, 4)
                        base = b * TB
                        lgMs = []
                        for hh in range(4):
                            ro = hh * 32
                            qh = qT[ro:ro + 32, g, base:base + TB]
                            lg = plgM.tile([128, 512], f32)
                            nc.tensor.matmul(
                                lg[:, 0:256], kT[ro:ro + 32, g, base:base + 128],
                                qh, start=True, stop=True, tile_position=(ro, 0),
                            )
                            nc.tensor.matmul(
                                lg[:, 256:512],
                                kT[ro:ro + 32, g, base + 128:base + 256],
                                qh, start=True, stop=True, tile_position=(ro, 0),
                            )
                            lgMs.append(lg)
                        lgT = plgT.tile([128, 512], f32)
                        for hh in range(4):
                            ro = hh * 32
                            qh = qT[ro:ro + 32, g, base:base + TB]
                            nc.tensor.matmul(
                                lgT[ro:ro + 32, 0:256],
                                kT[ro:ro + 32, g, base + 256:base + 288],
                                qh, start=True, stop=True, tile_position=(ro, ro),
                            )
                            nc.tensor.matmul(
                                lgT[ro:ro + 32, 256:512],
                                kbuf_sb[ro:ro + 32, g, :],
                                qh, start=True, stop=True, tile_position=(ro, ro),
                            )
                        exMs = []
                        for hh in range(4):
                            nc.vector.tensor_tensor(
                                lgMs[hh][:], lgMs[hh][:], mask_sb[:, b, 0:512], ADD
                            )
                            ex = exp_pool.tile([128, 512], bf16)
                            nc.scalar.activation(ex[:], lgMs[hh][:], EXP)
                            exMs.append(ex)
                        nc.vector.tensor_tensor(
                            lgT[:], lgT[:], mask_sb[:, b, 512:1024], ADD
                        )
                        exT = exp_pool.tile([128, 512], bf16)
                        nc.scalar.activation(exT[:], lgT[:], EXP)
                        cur = (b, g, base, exMs, exT)
                    else:
                        cur = None
                    if prev is not None:
                        b, g, base, exMs, exT = prev
                        for hh in range(4):
                            h = g * 4 + hh
                            ro = hh * 32
                            hc = h * 65
                            ov = pov.tile([128, 256], f32)
                            nc.tensor.matmul(
                                ov[0:65, :], vA[:, 2 * b, hc:hc + 65],
                                exMs[hh][:, 0:256], start=True, stop=False,
                            )
                            nc.tensor.matmul(
                                ov[0:65, :], vA[:, 2 * b + 1, hc:hc + 65],
                                exMs[hh][:, 256:512], start=False, stop=False,
                            )
                            nc.tensor.matmul(
                                ov[0:65, :], vTail[ro:ro + 32, b, hc:hc + 65],
                                exT[ro:ro + 32, 0:256], start=False, stop=False,
                                tile_position=(ro, 0),
                            )
                            nc.tensor.matmul(
                                ov[0:65, :], vbuf_sb[ro:ro + 32, hc:hc + 65],
                                exT[ro:ro + 32, 256:512], start=False, stop=True,
                                tile_position=(ro, 0),
                            )
                            od = oT[(h % 2) * 64:(h % 2) * 64 + 64, h // 2,
                                    base:base + TB]
                            nc.vector.tensor_copy(od, ov[0:64, :])
                            idx = b * H + h
                            # engine copies need 32-aligned partition offsets,
                            # and DMA cannot read PSUM: stage the sum row at
                            # partition 0, then DMA it to its s_all row
                            sst = s_pool.tile([1, TB], f32)
                            nc.scalar.copy(sst[:], ov[64:65, :])
                            nc.sync.dma_start(s_all[idx:idx + 1, :], sst[:])
                    prev = cur

            # ---- softmax normalization: one batched reciprocal, one
            # partition->free transposing DMA to land all recip rows on
            # partition 0, then rank-1 broadcasts (ones (x) r) into oT
            # partition->free transpose via a DRAM bounce (SBUF APs cannot
            # express a 32-partition read landing on one partition)
            nc.vector.reciprocal(r_all[:], s_all[:])
            nc.sync.dma_start(d_rsc[:], r_all[:])
            nc.sync.dma_start(
                r_flat[0:1, :], d_rsc[:].rearrange("a f -> (a f)")[None, :]
            )
            with (
                tc.tile_pool(name="prb", bufs=4, space="PSUM") as prb,
                tc.tile_pool(name="pp3", bufs=2, space="PSUM") as pp3,
            ):
                for b in range(NB):
                    for h in range(H):
                        idx = b * H + h
                        rb = prb.tile([64, TB], f32)
                        nc.tensor.matmul(
                            rb[:], ones[0:1, 0:64],
                            r_flat[0:1, idx * TB:(idx + 1) * TB].bitcast(f32r),
                            start=True, stop=True,
                        )
                        sl = oT[(h % 2) * 64:(h % 2) * 64 + 64, h // 2,
                                b * TB:(b + 1) * TB]
                        nc.vector.tensor_tensor(sl, sl, rb[:], MUL)
                    # out-projection for this block's token chunks
                    # (y[tok, :] = oT.T @ wp (+ bp)) overlaps block b+1's
                    # normalization multiplies
                    for m in (2 * b, 2 * b + 1):
                        for f in range(2):
                            pa = pp3.tile([128, 512], f32)
                            for c in range(8):
                                nc.tensor.matmul(
                                    pa[:], oT[:, c, m * 128:(m + 1) * 128],
                                    wp_sb[:, f, c, :], start=(c == 0),
                                    stop=(c == 7 and not with_bias),
                                )
                            if with_bias:
                                nc.tensor.matmul(
                                    pa[:], ones[0:1, 0:128],
                                    bp_sb[0:1, f * 512:(f + 1) * 512],
                                    start=False, stop=True,
                                )
                            ot = out_pool.tile([128, 512], f32)
                            nc.scalar.copy(ot[:], pa[:])
                            nc.sync.dma_start(
                                d_y[m * 128:(m + 1) * 128,
                                    f * 512:(f + 1) * 512], ot[:]
                            )
    return nc


def _get_runner(with_bias=True):
    key = ("runner", with_bias)
    if key in _CACHE:
        return _CACHE[key]
    import jax
    import concourse.mybir as mybir
    from concourse import bass2jax
    from jax.sharding import Mesh, PartitionSpec
    from jax.experimental.shard_map import shard_map

    nc = _build_nc(with_bias)
    bass2jax.install_neuronx_cc_hook()
    partition_name = nc.partition_id_tensor.name if nc.partition_id_tensor else None
    in_names, out_names, out_avals, out_shapes = [], [], [], []
    for alloc in nc.m.functions[0].allocations:
        if not isinstance(alloc, mybir.MemoryLocationSet):
            continue
        name = alloc.memorylocations[0].name
        if alloc.kind == "ExternalInput":
            if name != partition_name:
                in_names.append(name)
        elif alloc.kind == "ExternalOutput":
            shape = tuple(alloc.tensor_shape)
            dtype = mybir.dt.np(alloc.dtype)
            out_names.append(name)
            out_avals.append(jax.core.ShapedArray(shape, dtype))
            out_shapes.append((shape, dtype))
    n_params = len(in_names)
    n_outs = len(out_avals)
    all_in_names = in_names + out_names + ([partition_name] if partition_name else [])
    donate = tuple(range(n_params, n_params + n_outs))

    def _body(*args):
        operands = list(args)
        if partition_name is not None:
            operands.append(bass2jax.partition_id_tensor())
        outs = bass2jax._bass_exec_p.bind(
            *operands,
            out_avals=tuple(out_avals),
            in_names=tuple(all_in_names),
            out_names=tuple(out_names),
            lowering_input_output_aliases=(),
            sim_require_finite=True,
            sim_require_nnan=True,
            nc=nc,
        )
        return tuple(outs)

    devices = jax.devices()[:N_CORES]
    mesh = Mesh(np.asarray(devices), ("core",))
    sharded = jax.jit(
        shard_map(
            _body, mesh=mesh,
            in_specs=(PartitionSpec("core"),) * (n_params + n_outs),
            out_specs=(PartitionSpec("core"),) * n_outs,
            check_rep=False,
        ),
        donate_argnums=donate,
        keep_unused=True,
    )

    def run(in_maps):
        per_core = [[np.asarray(m[name]) for name in in_names] for m in in_maps]
        concat_in = [
            np.concatenate([per_core[c][i] for c in range(N_CORES)], axis=0)
            for i in range(n_params)
        ]
        concat_zeros = [
            np.zeros((N_CORES * s[0], *s[1:]), d) for (s, d) in out_shapes
        ]
        out_arrs = sharded(*concat_in, *concat_zeros)
        return [
            {
                name: np.asarray(out_arrs[i]).reshape(N_CORES, *out_shapes[i][0])[c]
                for i, name in enumerate(out_names)
            }
            for c in range(N_CORES)
        ]

    _CACHE[key] = run
    return run


# ------------------------------------------------------------------- host
def _prep_inputs(x, Wkv, bkv, Wq, bq, Wp, bp, buffer, sample_lengths):
    import ml_dtypes

    bfl = ml_dtypes.bfloat16
    x = np.asarray(x, np.float32)
    Wkv = np.asarray(Wkv, np.float32)
    bkv = np.asarray(bkv, np.float32)
    Wq = np.asarray(Wq, np.float32)
    bq = np.asarray(bq, np.float32)
    Wp = np.asarray(Wp, np.float32)
    bp = np.asarray(bp, np.float32)
    buffer = np.asarray(buffer, np.float32)
    lengths = np.asarray(sample_lengths).astype(np.int64)

    scale = 1.0 / math.sqrt(DK)
    starts = np.concatenate([[0], np.cumsum(lengths)[:-1]]).astype(np.int64)
    t = np.arange(T)
    seg = np.searchsorted(starts, t, side="right") - 1
    j = t - starts[seg]

    # weights pre-rearranged into exact SBUF layouts ([p, ...] partition-major)
    wkT = np.ascontiguousarray(Wkv[:KD, :].T)                       # [E, KD]
    wk_h = wkT.reshape(8, 128, 4, 128).transpose(1, 2, 0, 3).astype(bfl)
    wqT = np.ascontiguousarray(Wq.T * scale)                        # [E, KD]
    wq_h = wqT.reshape(8, 128, 4, 128).transpose(1, 2, 0, 3).astype(bfl)
    wv_aug = np.zeros((E, H, 65), np.float32)
    wv_aug[:, :, :64] = Wkv[KD:, :].T.reshape(E, H, DV)
    wv_h = (
        wv_aug.reshape(E, H * 65).reshape(8, 128, 4, 260)
        .transpose(1, 2, 0, 3).astype(bfl)
    )
    wpT = np.ascontiguousarray(Wp.T)                                # [E, E]
    wp_h = wpT.reshape(8, 128, 2, 512).transpose(1, 2, 0, 3).astype(bfl)

    bk2 = np.ascontiguousarray(bkv[:KD].reshape(4, 128).T)
    bq2 = np.ascontiguousarray((bq * scale).reshape(4, 128).T)
    bv_aug = np.zeros((H, 65), np.float32)
    bv_aug[:, :64] = bkv[KD:].reshape(H, DV)
    bv_aug[:, 64] = 1.0
    bv_row = np.ascontiguousarray(bv_aug.reshape(1, H * 65))
    bp_row = np.ascontiguousarray(bp[None, :])
    ones_row = np.ones((1, 128), np.float32)

    kbufT = np.zeros((KD, 32), np.float32)
    kbufT[:, :HALO] = buffer[:, :KD].T
    kbuf_h = kbufT.reshape(4, 128, 32).transpose(1, 0, 2).astype(bfl)
    vbuf = np.zeros((32, H * 65), np.float32)
    vb = vbuf.reshape(32, H, 65)
    vb[:HALO, :, :64] = buffer[:, KD:].reshape(HALO, H, DV)
    vb[:HALO, :, 64] = 1.0
    vbuf4_h = np.tile(vbuf, (4, 1)).astype(bfl)

    xTp = np.zeros((E, T + HALO + 33), np.float32)
    xTp[:, HALO:HALO + T] = x.T

    in_maps = []
    for c in range(N_CORES):
        t0 = c * SHARD
        xT_c = np.ascontiguousarray(
            xTp[:, t0:t0 + NTOK].reshape(8, 128, NTOK).transpose(1, 0, 2)
        ).astype(bfl)
        mask = np.full((128, NB, 1024), NEG, np.float32)
        for bblk in range(NB):
            i = np.arange(TB)
            tt = t0 + bblk * TB + i
            st = starts[seg[tt]]
            jj = j[tt]
            for r in range(2):
                p = np.arange(128)[:, None]
                g = t0 - HALO + bblk * TB + r * 128 + p
                valid = (
                    (g >= tt[None, :] - HALO) & (g <= tt[None, :])
                    & (g >= st[None, :]) & (g >= 0) & (g < T)
                )
                mask[:, bblk, r * 256:(r + 1) * 256] = np.where(valid, 0.0, NEG)
            p32 = np.arange(32)[:, None]
            g = t0 - HALO + bblk * TB + 256 + p32
            valid = (
                (g >= tt[None, :] - HALO) & (g <= tt[None, :])
                & (g >= st[None, :]) & (g >= 0) & (g < T)
            )
            tailm = np.where(valid, 0.0, NEG)
            pb = np.arange(32)[:, None]
            validb = (pb >= jj[None, :]) & (pb <= HALO - 1)
            bufm = np.where(validb, 0.0, NEG)
            for rr in range(4):
                mask[rr * 32:(rr + 1) * 32, bblk, 512:768] = tailm
                mask[rr * 32:(rr + 1) * 32, bblk, 768:1024] = bufm
        in_maps.append({
            "xT": xT_c, "wk": wk_h, "wq": wq_h, "wv": wv_h, "wp": wp_h,
            "kbufT": kbuf_h, "vbuf4": vbuf4_h, "ones": ones_row,
            "mask": np.ascontiguousarray(mask),
            "bk": bk2, "bq": bq2, "bv": bv_row, "bp": bp_row,
        })
    return in_maps, seg, j


def kernel(x, Wkv, bkv, Wq, bq, Wp, bp, buffer, sample_lengths):
    in_maps, seg, j = _prep_inputs(
        x, Wkv, bkv, Wq, bq, Wp, bp, buffer, sample_lengths
    )
    with_bias = bool(
        np.any(np.asarray(bkv)) or np.any(np.asarray(bq)) or np.any(np.asarray(bp))
    )
    run = _get_runner(with_bias)
    results = run(in_maps)
    out_full = np.concatenate([results[c]["yout"] for c in range(N_CORES)], axis=0)
    y = np.zeros((B, MAXL, E), np.float32)
    ok = j < MAXL
    y[seg[ok], j[ok]] = out_full[ok]
    return y


# revision 26
# speedup vs baseline: 1.4658x; 1.0941x over previous
"""Trainium2 Bass kernel for nn_DistiledMultiheadAttention_76476187673064.

Sliding-window (W=32) single-query attention over ragged sequences with a
learned pre-context buffer, plus input/output projections.

Strategy (8 NeuronCores, data-parallel over flat tokens):
  - Each core owns 512 tokens; kv for a 31-token halo is recomputed locally
    (plus one masked pad column), so no collectives are needed.
  - All matmul operands are bf16 (host-cast, fp32 PSUM accumulation):
    halves HBM traffic and LDWEIGHTS time, and removes the f32r
    narrow-output rate penalty.
  - Host passes pre-rearranged weights/activations so every DMA is a
    straight [128, N] partition-major copy (one big descriptor per
    partition) and every matmul's contraction lands on SBUF partitions:
      * K/Q projections feature-major (kT, qT: [feat, tok])
      * V projection token-major, augmented with a ones column per head
        (so PV emits per-head softmax sums for free)
      * QK logits ctx-major [ctx, tok]; the tail+buffer columns of 4 heads
        are packed into one full 128-partition PSUM tile (1/4 the mask/exp
        work); band+segment+buffer masking via a host-precomputed additive
        mask; exp without max-subtraction (logits are bounded)
  - ScalarE runs ONLY Exp + Copy (both in one activation table -> a single
    ACT_TABLE_LOAD for the whole kernel; the baseline's Exp<->Reciprocal
    alternation cost 32 table loads = 41us).  All 32 softmax-sum rows are
    gathered into one [32, 256] tile and reciprocated by a single DVE
    InstReciprocal; normalization is applied as rank-1 broadcast matmuls
    (ones (x) recip row) multiplied into the attention output.
"""
import math
import sys

sys.path.insert(0, "/opt/trn_rl_repo")

import numpy as np

# ---------------------------------------------------------------- constants
T = 4096
E = 1024
KD = 512          # key dim
H = 16            # heads
W = 32            # window
DK = KD // H      # 32
DV = E // H       # 64
B = 8
MAXL = 768
N_CORES = 8
SHARD = T // N_CORES          # 512 tokens per core
HALO = W - 1                  # 31
NTOK = SHARD + HALO + 1       # 544 token columns incl. halo + 1 pad
TB = 256                      # attention token block
NB = SHARD // TB              # 2 blocks per core
NEG = -30000.0

_CACHE = {}


# ------------------------------------------------------------- tile patches
def _apply_tile_patches():
    """This container's walrus only supports ONE sync-wait per instruction;
    redistribute extra Tile-assigned waits onto single-wait InstNoOp carriers."""
    import concourse.mybir as mybir
    import concourse.tile as tile
    from concourse.vector_clock import ScopedClock

    if getattr(tile.TileContext, "_wait_split_patched", False):
        return
    orig_commit = tile.TileContext._commit_and_lower

    def commit_split(self, inst, original_block, old_bb_map, bb_to_exit_bb):
        si = getattr(inst, "sync_info", None)
        if si is not None and si.on_wait and len(si.on_wait) > 1:
            engine = inst.engine
            if engine is not None and engine != mybir.EngineType.Unassigned:
                waits = list(si.on_wait)
                si.on_wait = waits[-1:]
                for w in waits[:-1]:
                    noop = mybir.InstNoOp(
                        name=self.nc.get_next_instruction_name(),
                        sync_info=mybir.SyncInfo(on_wait=[w], on_update=[]),
                        bass_nofuse=True,
                        engine=engine,
                        text_hint="wait_split",
                    )
                    orig_commit(self, noop, original_block, old_bb_map, bb_to_exit_bb)
        return orig_commit(self, inst, original_block, old_bb_map, bb_to_exit_bb)

    def drain_and_barrier(self, tick_clock, wait_clock):
        drain_inst = self.nc.sync.drain()
        wait_clock.add_sem_waits(
            drain_inst.ins, ScopedClock({None: tick_clock.global_clock})
        )
        si = drain_inst.ins.sync_info
        if si is not None and si.on_wait and len(si.on_wait) > 1:
            waits = list(si.on_wait)
            si.on_wait = waits[:1]
            for w in waits[1:]:
                nop = self.nc.sync.nop(nofuse=True)
                nsi = nop.ins.sync_info
                if nsi is None:
                    nop.ins.sync_info = mybir.SyncInfo(on_wait=[w], on_update=[])
                else:
                    nsi.on_wait = list(nsi.on_wait or []) + [w]
        self.nc.all_engine_barrier()
        assert self.sems is not None
        popped = self.nc._tile_sem_poison_stack.pop()
        assert popped is self._sem_poison
        self.nc.clear_and_free_semaphores(list(self.sems.allocated().values()))
        self.nc.all_engine_barrier()

    tile.TileContext._commit_and_lower = commit_split
    tile.TileContext._drain_and_barrier = drain_and_barrier
    tile.TileContext._wait_split_patched = True


# ------------------------------------------------------------- device build
def _build_nc(with_bias=True):
    import concourse.bass as bass
    import concourse.mybir as mybir
    import concourse.tile as tile

    _apply_tile_patches()
    f32 = mybir.dt.float32
    f32r = mybir.dt.float32r
    bf16 = mybir.dt.bfloat16
    ADD = mybir.AluOpType.add
    MUL = mybir.AluOpType.mult
    EXP = mybir.ActivationFunctionType.Exp

    nc = bass.Bass()
    d_xT = nc.dram_tensor("xT", [128, 8, NTOK], bf16, kind="ExternalInput")
    d_wk = nc.dram_tensor("wk", [128, 4, 8, 128], bf16, kind="ExternalInput")
    d_wq = nc.dram_tensor("wq", [128, 4, 8, 128], bf16, kind="ExternalInput")
    d_wv = nc.dram_tensor("wv", [128, 4, 8, 260], bf16, kind="ExternalInput")
    d_wp = nc.dram_tensor("wp", [128, 2, 8, 512], bf16, kind="ExternalInput")
    d_kbufT = nc.dram_tensor("kbufT", [128, 4, 32], bf16, kind="ExternalInput")
    d_vbuf4 = nc.dram_tensor("vbuf4", [128, 1040], bf16, kind="ExternalInput")
    d_ones = nc.dram_tensor("ones", [1, 128], f32r, kind="ExternalInput")
    d_mask = nc.dram_tensor("mask", [128, NB, 1024], f32, kind="ExternalInput")
    d_bk = nc.dram_tensor("bk", [128, 4], f32, kind="ExternalInput")
    d_bq = nc.dram_tensor("bq", [128, 4], f32, kind="ExternalInput")
    d_bv = nc.dram_tensor("bv", [1, H * 65], f32r, kind="ExternalInput")
    d_bp = nc.dram_tensor("bp", [1, E], f32r, kind="ExternalInput")
    d_sel2 = nc.dram_tensor("sel2", [2, 128], f32r, kind="ExternalInput")
    d_y = nc.dram_tensor("yout", [SHARD, E], f32, kind="ExternalOutput")
    d_rsc = nc.dram_tensor("rscratch", [32, TB], f32, kind="Internal")
    d_ssc = nc.dram_tensor("sscratch", [1, 32 * TB], f32, kind="Internal")

    with tile.TileContext(nc) as tc, nc.allow_low_precision(
        reason="bf16 matmul operands; fp32 PSUM accumulation throughout"
    ):
        with (
            tc.tile_pool(name="x", bufs=1) as x_pool,
            tc.tile_pool(name="wgt", bufs=1) as w_pool,
            tc.tile_pool(name="const", bufs=1) as const_pool,
            tc.tile_pool(name="kqv", bufs=1) as kqv_pool,
            tc.tile_pool(name="exp", bufs=12) as exp_pool,
            tc.tile_pool(name="rrow", bufs=4) as r_pool,
            tc.tile_pool(name="srow", bufs=4) as s_pool,
            tc.tile_pool(name="out", bufs=3) as out_pool,
        ):
            # ---- x first (feature-major, all tokens incl. halo+pad)
            xT = x_pool.tile([128, 8, NTOK], bf16)
            nc.sync.dma_start(xT[:], d_xT[:])

            # ---- weights as whole tiles, in consumption order
            wk_sb = w_pool.tile([128, 4, 8, 128], bf16)
            nc.sync.dma_start(wk_sb[:], d_wk[:])
            wq_sb = w_pool.tile([128, 4, 8, 128], bf16)
            nc.sync.dma_start(wq_sb[:], d_wq[:])
            wv_sb = w_pool.tile([128, 4, 8, 260], bf16)
            nc.sync.dma_start(wv_sb[:], d_wv[:])

            # ---- attention constants
            kbuf_sb = const_pool.tile([128, 4, 32], bf16)
            nc.sync.dma_start(kbuf_sb[:], d_kbufT[:])
            vbuf_sb = const_pool.tile([128, 1040], bf16)
            nc.sync.dma_start(vbuf_sb[:], d_vbuf4[:])
            ones = const_pool.tile([1, 128], f32r)
            nc.sync.dma_start(ones[:], d_ones[:])
            sel2 = const_pool.tile([2, 128], f32r)
            nc.sync.dma_start(sel2[:], d_sel2[:])
            mask_sb = const_pool.tile([128, NB, 1024], f32)
            nc.sync.dma_start(mask_sb[:], d_mask[:])
            if with_bias:
                bk_sb = const_pool.tile([128, 4], f32)
                nc.sync.dma_start(bk_sb[:], d_bk[:])
                bq_sb = const_pool.tile([128, 4], f32)
                nc.sync.dma_start(bq_sb[:], d_bq[:])
                bv_sb = const_pool.tile([1, H * 65], f32r)
                nc.sync.dma_start(bv_sb[:], d_bv[:])
                bp_sb = const_pool.tile([1, E], f32r)
                nc.sync.dma_start(bp_sb[:], d_bp[:])

            # ---- output-projection weights last (consumed last)
            wp_sb = w_pool.tile([128, 2, 8, 512], bf16)
            nc.sync.dma_start(wp_sb[:], d_wp[:])

            # ---- persistent activations
            kT = kqv_pool.tile([128, 4, NTOK], bf16)    # K feature-major
            qT = kqv_pool.tile([128, 4, SHARD], bf16)   # Q feature-major (scaled)
            vA = kqv_pool.tile([128, 5, H * 65], bf16)  # V token-major + ones col
            vTail = kqv_pool.tile([128, NB, H * 65], bf16)  # tail-ctx V, 4x replicated
            oT = kqv_pool.tile([128, 8, SHARD], bf16)   # attention out feature-major
            s_half = [kqv_pool.tile([16, TB], f32, name=f"s_half{i}")
                      for i in range(NB)]  # sums
            r_half = [kqv_pool.tile([16, TB], f32, name=f"r_half{i}")
                      for i in range(NB)]  # recips
            # recips rearranged: row 0 = even heads, row 1 = odd heads
            r2 = kqv_pool.tile([2, 16 * TB], f32)

            with tc.tile_pool(name="pp1", bufs=2, space="PSUM") as pp1:
                # K projection: kT[:, m, :] = wk[:, mchunk].T @ xT (+ bk)
                for m in range(4):
                    pa = pp1.tile([128, 512], f32, tag="pa")
                    pb = pp1.tile([128, 32], f32, tag="pb")
                    for e in range(8):
                        nc.tensor.matmul(
                            pa[:], wk_sb[:, m, e, :], xT[:, e, 0:512],
                            start=(e == 0), stop=(e == 7),
                        )
                        nc.tensor.matmul(
                            pb[:], wk_sb[:, m, e, :], xT[:, e, 512:NTOK],
                            start=(e == 0), stop=(e == 7),
                        )
                    if with_bias:
                        nc.scalar.add(kT[:, m, 0:512], pa[:], bk_sb[:, m:m + 1])
                        nc.scalar.add(kT[:, m, 512:NTOK], pb[:], bk_sb[:, m:m + 1])
                    else:
                        nc.scalar.copy(kT[:, m, 0:512], pa[:])
                        nc.scalar.copy(kT[:, m, 512:NTOK], pb[:])

                # Q projection (tokens only, no halo)
                for m in range(4):
                    pa = pp1.tile([128, 512], f32, tag="pa")
                    for e in range(8):
                        nc.tensor.matmul(
                            pa[:], wq_sb[:, m, e, :], xT[:, e, HALO:HALO + SHARD],
                            start=(e == 0), stop=(e == 7),
                        )
                    if with_bias:
                        nc.scalar.add(qT[:, m, :], pa[:], bq_sb[:, m:m + 1])
                    else:
                        nc.scalar.copy(qT[:, m, :], pa[:])

                # V projection token-major (wv pre-augmented with zero ones-cols):
                # vA[tok, h*65:h*65+65] = [x @ Wv_h.T + bv_h | 1]
                tok_sizes = [128, 128, 128, 128, 32]
                for f in range(4):
                    for i in range(5):
                        mt = tok_sizes[i]
                        pa = pp1.tile([128, 260], f32, tag="pv")
                        for e in range(8):
                            nc.tensor.matmul(
                                pa[0:mt, :],
                                xT[:, e, i * 128:i * 128 + mt],
                                wv_sb[:, f, e, :],
                                start=(e == 0), stop=(e == 7 and not with_bias),
                            )
                        if with_bias:
                            nc.tensor.matmul(
                                pa[0:mt, :], ones[0:1, 0:mt],
                                bv_sb[0:1, f * 260:(f + 1) * 260],
                                start=False, stop=True,
                            )
                        nc.vector.tensor_copy(
                            vA[0:mt, i, f * 260:(f + 1) * 260], pa[0:mt, :]
                        )
                        if not with_bias:
                            # ones columns via strided add (psum zeros there)
                            ov_view = vA[0:mt, i, f * 260:(f + 1) * 260].rearrange(
                                "p (h c) -> p h c", c=65
                            )[:, :, 64:65]
                            nc.gpsimd.tensor_scalar_add(ov_view, ov_view, 1.0)

            # replicate the per-block tail-ctx V rows across all four
            # 32-partition groups so packed-tail PV matmuls line up
            for b in range(NB):
                for r in range(4):
                    nc.sync.dma_start(
                        vTail[r * 32:(r + 1) * 32, b, :], vA[0:32, 2 * b + 2, :]
                    )

            # ---- attention
            with (
                tc.tile_pool(name="plgM", bufs=4, space="PSUM") as plgM,
                tc.tile_pool(name="plgT", bufs=2, space="PSUM") as plgT,
                tc.tile_pool(name="pov", bufs=2, space="PSUM") as pov,
            ):
                def emit_recip_chain(b):
                    # gather this block's 16 sum rows from the DRAM scratch,
                    # reciprocate once on DVE, and bounce back through DRAM
                    # into the paired layout (row 0 = even heads, row 1 = odd)
                    half = 16 * TB
                    nc.sync.dma_start(
                        s_half[b][:],
                        d_ssc[0:1, b * half:(b + 1) * half].rearrange(
                            "p (a f) -> (p a) f", a=16
                        ),
                    )
                    nc.vector.reciprocal(r_half[b][:], s_half[b][:])
                    nc.sync.dma_start(d_rsc[b * 16:(b + 1) * 16, :], r_half[b][:])
                    nc.sync.dma_start(
                        r2[0:2, b * 8 * TB:(b + 1) * 8 * TB].rearrange(
                            "p (a f) -> p a f", f=TB
                        ),
                        d_rsc[b * 16:(b + 1) * 16, :].rearrange(
                            "(a p) f -> p a f", p=2
                        ),
                    )

                prev = None
                for it in range(NB * 4 + 1):
                    if it < NB * 4:
                        b, g = divmod(it, 4)
                        base = b * TB
                    cur_exMs, cur_ov = [], None
                    lgT = None
                    for hh in range(4):
                        if it < NB * 4:
                            # QK + mask + exp for this head, emitted per-head
                            # so the DVE/ACT chain pipelines behind the PE
                            ro = hh * 32
                            qh = qT[ro:ro + 32, g, base:base + TB]
                            lg = plgM.tile([128, 512], f32)
                            nc.tensor.matmul(
                                lg[:, 0:256], kT[ro:ro + 32, g, base:base + 128],
                                qh, start=True, stop=True, tile_position=(ro, 0),
                            )
                            nc.tensor.matmul(
                                lg[:, 256:512],
                                kT[ro:ro + 32, g, base + 128:base + 256],
                                qh, start=True, stop=True, tile_position=(ro, 0),
                            )
                            if lgT is None:
                                lgT = plgT.tile([128, 512], f32)
                            nc.tensor.matmul(
                                lgT[ro:ro + 32, 0:256],
                                kT[ro:ro + 32, g, base + 256:base + 288],
                                qh, start=True, stop=True, tile_position=(ro, ro),
                            )
                            nc.tensor.matmul(
                                lgT[ro:ro + 32, 256:512],
                                kbuf_sb[ro:ro + 32, g, :],
                                qh, start=True, stop=True, tile_position=(ro, ro),
                            )
                            nc.vector.tensor_tensor(
                                lg[:], lg[:], mask_sb[:, b, 0:512], ADD
                            )
                            ex = exp_pool.tile([128, 512], bf16)
                            nc.scalar.activation(ex[:], lg[:], EXP)
                            cur_exMs.append(ex)
                        if prev is not None:
                            # previous group's PV for this head slot, two heads
                            # per [128, 512] PSUM tile (sum rows evicted in one
                            # [1, 512] copy per pair)
                            pb, pg, pbase, pexMs, pexT = prev
                            h = pg * 4 + hh
                            ro = hh * 32
                            hc = h * 65
                            if hh % 2 == 0:
                                cur_ov = pov.tile([128, 512], f32)
                            ovr = cur_ov[:, (hh % 2) * 256:(hh % 2) * 256 + 256]
                            nc.tensor.matmul(
                                ovr[0:65, :], vA[:, 2 * pb, hc:hc + 65],
                                pexMs[hh][:, 0:256], start=True, stop=False,
                            )
                            nc.tensor.matmul(
                                ovr[0:65, :], vA[:, 2 * pb + 1, hc:hc + 65],
                                pexMs[hh][:, 256:512], start=False, stop=False,
                            )
                            nc.tensor.matmul(
                                ovr[0:65, :], vTail[ro:ro + 32, pb, hc:hc + 65],
                                pexT[ro:ro + 32, 0:256], start=False, stop=False,
                                tile_position=(ro, 0),
                            )
                            nc.tensor.matmul(
                                ovr[0:65, :], vbuf_sb[ro:ro + 32, hc:hc + 65],
                                pexT[ro:ro + 32, 256:512], start=False, stop=True,
                                tile_position=(ro, 0),
                            )
                            od = oT[(h % 2) * 64:(h % 2) * 64 + 64, h // 2,
                                    pbase:pbase + TB]
                            if hh < 2:
                                nc.scalar.copy(od, ovr[0:64, :])
                            else:
                                nc.vector.tensor_copy(od, ovr[0:64, :])
                            if hh % 2 == 1:
                                idx = pb * H + (h - 1)
                                sst = s_pool.tile([1, 2 * TB], f32)
                                if hh < 2:
                                    nc.vector.tensor_copy(sst[:], cur_ov[64:65, :])
                                else:
                                    nc.scalar.copy(sst[:], cur_ov[64:65, :])
                                nc.sync.dma_start(
                                    d_ssc[0:1, idx * TB:(idx + 2) * TB], sst[:]
                                )
                    if it < NB * 4:
                        nc.vector.tensor_tensor(
                            lgT[:], lgT[:], mask_sb[:, b, 512:1024], ADD
                        )
                        exT = exp_pool.tile([128, 512], bf16)
                        nc.scalar.activation(exT[:], lgT[:], EXP)
                        prev = (b, g, base, cur_exMs, exT)
                    else:
                        prev = None
                    if it == 4:
                        emit_recip_chain(0)
                emit_recip_chain(1)

            # ---- softmax normalization (rank-2 selector broadcast: one
            # matmul + one multiply covers a head pair) + output projection
            with (
                tc.tile_pool(name="prb", bufs=4, space="PSUM") as prb,
                tc.tile_pool(name="pp3", bufs=2, space="PSUM") as pp3,
            ):
                for b in range(NB):
                    for c in range(8):
                        rb2 = prb.tile([128, TB], f32)
                        nc.tensor.matmul(
                            rb2[:], sel2[:],
                            r2[0:2, (b * 8 + c) * TB:(b * 8 + c + 1) * TB]
                            .bitcast(f32r),
                            start=True, stop=True,
                        )
                        sl = oT[:, c, b * TB:(b + 1) * TB]
                        nc.vector.tensor_tensor(sl, sl, rb2[:], MUL)
                    # out-projection for this block's token chunks
                    # (y[tok, :] = oT.T @ wp (+ bp)) overlaps block b+1's
                    # normalization multiplies
                    for m in (2 * b, 2 * b + 1):
                        for f in range(2):
                            pa = pp3.tile([128, 512], f32)
                            for c in range(8):
                                nc.tensor.matmul(
                                    pa[:], oT[:, c, m * 128:(m + 1) * 128],
                                    wp_sb[:, f, c, :], start=(c == 0),
                                    stop=(c == 7 and not with_bias),
                                )
                            if with_bias:
                                nc.tensor.matmul(
                                    pa[:], ones[0:1, 0:128],
                                    bp_sb[0:1, f * 512:(f + 1) * 512],
                                    start=False, stop=True,
                                )
                            ot = out_pool.tile([128, 512], f32)
                            nc.scalar.copy(ot[:], pa[:])
                            nc.sync.dma_start(
                                d_y[m * 128:(m + 1) * 128,
                                    f * 512:(f + 1) * 512], ot[:]
                            )
    return nc


def _get_runner(with_bias=True):
    key = ("runner", with_bias)
    if key in _CACHE:
        return _CACHE[key]
    import jax
    import concourse.mybir as mybir
    from concourse import bass2jax
    from jax.sharding import Mesh, PartitionSpec
    from jax.experimental.shard_map import shard_map

    nc = _build_nc(with_bias)
    bass2jax.install_neuronx_cc_hook()
    partition_name = nc.partition_id_tensor.name if nc.partition_id_tensor else None
    in_names, out_names, out_avals, out_shapes = [], [], [], []
    for alloc in nc.m.functions[0].allocations:
        if not isinstance(alloc, mybir.MemoryLocationSet):
            continue
        name = alloc.memorylocations[0].name
        if alloc.kind == "ExternalInput":
            if name != partition_name:
                in_names.append(name)
        elif alloc.kind == "ExternalOutput":
            shape = tuple(alloc.tensor_shape)
            dtype = mybir.dt.np(alloc.dtype)
            out_names.append(name)
            out_avals.append(jax.core.ShapedArray(shape, dtype))
            out_shapes.append((shape, dtype))
    n_params = len(in_names)
    n_outs = len(out_avals)
    all_in_names = in_names + out_names + ([partition_name] if partition_name else [])
    donate = tuple(range(n_params, n_params + n_outs))

    def _body(*args):
        operands = list(args)
        if partition_name is not None:
            operands.append(bass2jax.partition_id_tensor())
        outs = bass2jax._bass_exec_p.bind(
            *operands,
            out_avals=tuple(out_avals),
            in_names=tuple(all_in_names),
            out_names=tuple(out_names),
            lowering_input_output_aliases=(),
            sim_require_finite=True,
            sim_require_nnan=True,
            nc=nc,
        )
        return tuple(outs)

    devices = jax.devices()[:N_CORES]
    mesh = Mesh(np.asarray(devices), ("core",))
    sharded = jax.jit(
        shard_map(
            _body, mesh=mesh,
            in_specs=(PartitionSpec("core"),) * (n_params + n_outs),
            out_specs=(PartitionSpec("core"),) * n_outs,
            check_rep=False,
        ),
        donate_argnums=donate,
        keep_unused=True,
    )

    def run(in_maps):
        per_core = [[np.asarray(m[name]) for name in in_names] for m in in_maps]
        concat_in = [
            np.concatenate([per_core[c][i] for c in range(N_CORES)], axis=0)
            for i in range(n_params)
        ]
        concat_zeros = [
            np.zeros((N_CORES * s[0], *s[1:]), d) for (s, d) in out_shapes
        ]
        out_arrs = sharded(*concat_in, *concat_zeros)
        return [
            {
                name: np.asarray(out_arrs[i]).reshape(N_CORES, *out_shapes[i][0])[c]
                for i, name in enumerate(out_names)
            }
            for c in range(N_CORES)
        ]

    _CACHE[key] = run
    return run


# ------------------------------------------------------------------- host
def _prep_inputs(x, Wkv, bkv, Wq, bq, Wp, bp, buffer, sample_lengths):
    import ml_dtypes

    bfl = ml_dtypes.bfloat16
    x = np.asarray(x, np.float32)
    Wkv = np.asarray(Wkv, np.float32)
    bkv = np.asarray(bkv, np.float32)
    Wq = np.asarray(Wq, np.float32)
    bq = np.asarray(bq, np.float32)
    Wp = np.asarray(Wp, np.float32)
    bp = np.asarray(bp, np.float32)
    buffer = np.asarray(buffer, np.float32)
    lengths = np.asarray(sample_lengths).astype(np.int64)

    scale = 1.0 / math.sqrt(DK)
    starts = np.concatenate([[0], np.cumsum(lengths)[:-1]]).astype(np.int64)
    t = np.arange(T)
    seg = np.searchsorted(starts, t, side="right") - 1
    j = t - starts[seg]

    # weights pre-rearranged into exact SBUF layouts ([p, ...] partition-major)
    wkT = np.ascontiguousarray(Wkv[:KD, :].T)                       # [E, KD]
    wk_h = wkT.reshape(8, 128, 4, 128).transpose(1, 2, 0, 3).astype(bfl)
    wqT = np.ascontiguousarray(Wq.T * scale)                        # [E, KD]
    wq_h = wqT.reshape(8, 128, 4, 128).transpose(1, 2, 0, 3).astype(bfl)
    wv_aug = np.zeros((E, H, 65), np.float32)
    wv_aug[:, :, :64] = Wkv[KD:, :].T.reshape(E, H, DV)
    wv_h = (
        wv_aug.reshape(E, H * 65).reshape(8, 128, 4, 260)
        .transpose(1, 2, 0, 3).astype(bfl)
    )
    wpT = np.ascontiguousarray(Wp.T)                                # [E, E]
    wp_h = wpT.reshape(8, 128, 2, 512).transpose(1, 2, 0, 3).astype(bfl)

    bk2 = np.ascontiguousarray(bkv[:KD].reshape(4, 128).T)
    bq2 = np.ascontiguousarray((bq * scale).reshape(4, 128).T)
    bv_aug = np.zeros((H, 65), np.float32)
    bv_aug[:, :64] = bkv[KD:].reshape(H, DV)
    bv_aug[:, 64] = 1.0
    bv_row = np.ascontiguousarray(bv_aug.reshape(1, H * 65))
    bp_row = np.ascontiguousarray(bp[None, :])
    ones_row = np.ones((1, 128), np.float32)
    sel2 = np.zeros((2, 128), np.float32)
    sel2[0, :64] = 1.0
    sel2[1, 64:] = 1.0

    kbufT = np.zeros((KD, 32), np.float32)
    kbufT[:, :HALO] = buffer[:, :KD].T
    kbuf_h = kbufT.reshape(4, 128, 32).transpose(1, 0, 2).astype(bfl)
    vbuf = np.zeros((32, H * 65), np.float32)
    vb = vbuf.reshape(32, H, 65)
    vb[:HALO, :, :64] = buffer[:, KD:].reshape(HALO, H, DV)
    vb[:HALO, :, 64] = 1.0
    vbuf4_h = np.tile(vbuf, (4, 1)).astype(bfl)

    xTp = np.zeros((E, T + HALO + 33), np.float32)
    xTp[:, HALO:HALO + T] = x.T

    in_maps = []
    for c in range(N_CORES):
        t0 = c * SHARD
        xT_c = np.ascontiguousarray(
            xTp[:, t0:t0 + NTOK].reshape(8, 128, NTOK).transpose(1, 0, 2)
        ).astype(bfl)
        mask = np.full((128, NB, 1024), NEG, np.float32)
        for bblk in range(NB):
            i = np.arange(TB)
            tt = t0 + bblk * TB + i
            st = starts[seg[tt]]
            jj = j[tt]
            for r in range(2):
                p = np.arange(128)[:, None]
                g = t0 - HALO + bblk * TB + r * 128 + p
                valid = (
                    (g >= tt[None, :] - HALO) & (g <= tt[None, :])
                    & (g >= st[None, :]) & (g >= 0) & (g < T)
                )
                mask[:, bblk, r * 256:(r + 1) * 256] = np.where(valid, 0.0, NEG)
            p32 = np.arange(32)[:, None]
            g = t0 - HALO + bblk * TB + 256 + p32
            valid = (
                (g >= tt[None, :] - HALO) & (g <= tt[None, :])
                & (g >= st[None, :]) & (g >= 0) & (g < T)
            )
            tailm = np.where(valid, 0.0, NEG)
            pb = np.arange(32)[:, None]
            validb = (pb >= jj[None, :]) & (pb <= HALO - 1)
            bufm = np.where(validb, 0.0, NEG)
            for rr in range(4):
                mask[rr * 32:(rr + 1) * 32, bblk, 512:768] = tailm
                mask[rr * 32:(rr + 1) * 32, bblk, 768:1024] = bufm
        in_maps.append({
            "xT": xT_c, "wk": wk_h, "wq": wq_h, "wv": wv_h, "wp": wp_h,
            "kbufT": kbuf_h, "vbuf4": vbuf4_h, "ones": ones_row, "sel2": sel2,
            "mask": np.ascontiguousarray(mask),
            "bk": bk2, "bq": bq2, "bv": bv_row, "bp": bp_row,
        })
    return in_maps, seg, j


def kernel(x, Wkv, bkv, Wq, bq, Wp, bp, buffer, sample_lengths):
    in_maps, seg, j = _prep_inputs(
        x, Wkv, bkv, Wq, bq, Wp, bp, buffer, sample_lengths
    )
    with_bias = bool(
        np.any(np.asarray(bkv)) or np.any(np.asarray(bq)) or np.any(np.asarray(bp))
    )
    run = _get_runner(with_bias)
    results = run(in_maps)
    out_full = np.concatenate([results[c]["yout"] for c in range(N_CORES)], axis=0)
    y = np.zeros((B, MAXL, E), np.float32)
    ok = j < MAXL
    y[seg[ok], j[ok]] = out_full[ok]
    return y


# revision 40
# speedup vs baseline: 1.5571x; 1.0623x over previous
"""Trainium2 Bass kernel for nn_DistiledMultiheadAttention_76476187673064.

Sliding-window (W=32) single-query attention over ragged sequences with a
learned pre-context buffer, plus input/output projections.

Strategy (8 NeuronCores, data-parallel over flat tokens):
  - Each core owns 512 tokens; kv for a 31-token halo is recomputed locally
    (plus one masked pad column), so no collectives are needed.
  - All matmul operands are bf16 (host-cast, fp32 PSUM accumulation):
    halves HBM traffic and LDWEIGHTS time, and removes the f32r
    narrow-output rate penalty.
  - Host passes pre-rearranged weights/activations so every DMA is a
    straight [128, N] partition-major copy (one big descriptor per
    partition) and every matmul's contraction lands on SBUF partitions:
      * K/Q projections feature-major (kT, qT: [feat, tok])
      * V projection token-major, augmented with a ones column per head
        (so PV emits per-head softmax sums for free)
      * QK logits ctx-major [ctx, tok]; the tail+buffer columns of 4 heads
        are packed into one full 128-partition PSUM tile (1/4 the mask/exp
        work); band+segment+buffer masking via a host-precomputed additive
        mask; exp without max-subtraction (logits are bounded)
  - ScalarE runs ONLY Exp + Copy (both in one activation table -> a single
    ACT_TABLE_LOAD for the whole kernel; the baseline's Exp<->Reciprocal
    alternation cost 32 table loads = 41us).  All 32 softmax-sum rows are
    gathered into one [32, 256] tile and reciprocated by a single DVE
    InstReciprocal; normalization is applied as rank-1 broadcast matmuls
    (ones (x) recip row) multiplied into the attention output.
"""
import math
import sys

sys.path.insert(0, "/opt/trn_rl_repo")

import numpy as np

# ---------------------------------------------------------------- constants
T = 4096
E = 1024
KD = 512          # key dim
H = 16            # heads
W = 32            # window
DK = KD // H      # 32
DV = E // H       # 64
B = 8
MAXL = 768
N_CORES = 8
SHARD = T // N_CORES          # 512 tokens per core
HALO = W - 1                  # 31
NTOK = SHARD + HALO + 1       # 544 token columns incl. halo + 1 pad
TB = 256                      # attention token block
NB = SHARD // TB              # 2 blocks per core
NEG = -30000.0

_CACHE = {}


# ------------------------------------------------------------- tile patches
def _apply_tile_patches():
    """This container's walrus only supports ONE sync-wait per instruction;
    redistribute extra Tile-assigned waits onto single-wait InstNoOp carriers."""
    import concourse.mybir as mybir
    import concourse.tile as tile
    from concourse.vector_clock import ScopedClock

    if getattr(tile.TileContext, "_wait_split_patched", False):
        return
    orig_commit = tile.TileContext._commit_and_lower

    def commit_split(self, inst, original_block, old_bb_map, bb_to_exit_bb):
        si = getattr(inst, "sync_info", None)
        if si is not None and si.on_wait and len(si.on_wait) > 1:
            engine = inst.engine
            if engine is not None and engine != mybir.EngineType.Unassigned:
                waits = list(si.on_wait)
                si.on_wait = waits[-1:]
                for w in waits[:-1]:
                    noop = mybir.InstNoOp(
                        name=self.nc.get_next_instruction_name(),
                        sync_info=mybir.SyncInfo(on_wait=[w], on_update=[]),
                        bass_nofuse=True,
                        engine=engine,
                        text_hint="wait_split",
                    )
                    orig_commit(self, noop, original_block, old_bb_map, bb_to_exit_bb)
        return orig_commit(self, inst, original_block, old_bb_map, bb_to_exit_bb)

    def drain_and_barrier(self, tick_clock, wait_clock):
        drain_inst = self.nc.sync.drain()
        wait_clock.add_sem_waits(
            drain_inst.ins, ScopedClock({None: tick_clock.global_clock})
        )
        si = drain_inst.ins.sync_info
        if si is not None and si.on_wait and len(si.on_wait) > 1:
            waits = list(si.on_wait)
            si.on_wait = waits[:1]
            for w in waits[1:]:
                nop = self.nc.sync.nop(nofuse=True)
                nsi = nop.ins.sync_info
                if nsi is None:
                    nop.ins.sync_info = mybir.SyncInfo(on_wait=[w], on_update=[])
                else:
                    nsi.on_wait = list(nsi.on_wait or []) + [w]
        self.nc.all_engine_barrier()
        assert self.sems is not None
        popped = self.nc._tile_sem_poison_stack.pop()
        assert popped is self._sem_poison
        self.nc.clear_and_free_semaphores(list(self.sems.allocated().values()))
        self.nc.all_engine_barrier()

    tile.TileContext._commit_and_lower = commit_split
    tile.TileContext._drain_and_barrier = drain_and_barrier
    tile.TileContext._wait_split_patched = True


# ------------------------------------------------------------- device build
def _build_nc(with_bias=True):
    import concourse.bass as bass
    import concourse.mybir as mybir
    import concourse.tile as tile

    _apply_tile_patches()
    f32 = mybir.dt.float32
    f32r = mybir.dt.float32r
    bf16 = mybir.dt.bfloat16
    ADD = mybir.AluOpType.add
    MUL = mybir.AluOpType.mult
    EXP = mybir.ActivationFunctionType.Exp

    nc = bass.Bass()
    d_xT = nc.dram_tensor("xT", [128, 8, NTOK], bf16, kind="ExternalInput")
    d_wk = nc.dram_tensor("wk", [128, 4, 8, 128], bf16, kind="ExternalInput")
    d_wq = nc.dram_tensor("wq", [128, 4, 8, 128], bf16, kind="ExternalInput")
    d_wv = nc.dram_tensor("wv", [128, 4, 8, 260], bf16, kind="ExternalInput")
    d_wp = nc.dram_tensor("wp", [128, 2, 8, 512], bf16, kind="ExternalInput")
    d_kbufT = nc.dram_tensor("kbufT", [128, 4, 32], bf16, kind="ExternalInput")
    d_vbuf4 = nc.dram_tensor("vbuf4", [128, 1040], bf16, kind="ExternalInput")
    d_ones = nc.dram_tensor("ones", [1, 128], f32r, kind="ExternalInput")
    d_mask = nc.dram_tensor("mask", [128, NB, 1024], f32, kind="ExternalInput")
    d_bk = nc.dram_tensor("bk", [128, 4], f32, kind="ExternalInput")
    d_bq = nc.dram_tensor("bq", [128, 4], f32, kind="ExternalInput")
    d_bv = nc.dram_tensor("bv", [1, H * 65], f32r, kind="ExternalInput")
    d_bp = nc.dram_tensor("bp", [1, E], f32r, kind="ExternalInput")
    d_sel2 = nc.dram_tensor("sel2", [2, 128], f32r, kind="ExternalInput")
    d_y = nc.dram_tensor("yout", [SHARD, E], f32, kind="ExternalOutput")
    d_rsc = [nc.dram_tensor(f"rscratch{i}", [16, TB], f32, kind="Internal")
             for i in range(NB)]
    d_ssc = [nc.dram_tensor(f"sscratch{i}", [1, 16 * TB], f32, kind="Internal")
             for i in range(NB)]

    with tile.TileContext(nc) as tc, nc.allow_low_precision(
        reason="bf16 matmul operands; fp32 PSUM accumulation throughout"
    ):
        with (
            tc.tile_pool(name="x", bufs=1) as x_pool,
            tc.tile_pool(name="wgt", bufs=1) as w_pool,
            tc.tile_pool(name="const", bufs=1) as const_pool,
            tc.tile_pool(name="kqv", bufs=1) as kqv_pool,
            tc.tile_pool(name="exp", bufs=12) as exp_pool,
            tc.tile_pool(name="rrow", bufs=4) as r_pool,
            tc.tile_pool(name="srow", bufs=4) as s_pool,
            tc.tile_pool(name="out", bufs=3) as out_pool,
        ):
            # ---- x first (feature-major, all tokens incl. halo+pad)
            xT = x_pool.tile([128, 8, NTOK], bf16)
            nc.sync.dma_start(xT[:], d_xT[:])

            # ---- weights as whole tiles, in consumption order
            wk_sb = w_pool.tile([128, 4, 8, 128], bf16)
            nc.sync.dma_start(wk_sb[:], d_wk[:])
            wq_sb = w_pool.tile([128, 4, 8, 128], bf16)
            nc.sync.dma_start(wq_sb[:], d_wq[:])
            wv_sb = w_pool.tile([128, 4, 8, 260], bf16)
            nc.sync.dma_start(wv_sb[:], d_wv[:])

            # ---- attention constants
            kbuf_sb = const_pool.tile([128, 4, 32], bf16)
            nc.sync.dma_start(kbuf_sb[:], d_kbufT[:])
            vbuf_sb = const_pool.tile([128, 1040], bf16)
            nc.sync.dma_start(vbuf_sb[:], d_vbuf4[:])
            ones = const_pool.tile([1, 128], f32r)
            nc.sync.dma_start(ones[:], d_ones[:])
            sel2 = const_pool.tile([2, 128], f32r)
            nc.sync.dma_start(sel2[:], d_sel2[:])
            mask_sb = const_pool.tile([128, NB, 1024], f32)
            nc.sync.dma_start(mask_sb[:], d_mask[:])
            if with_bias:
                bk_sb = const_pool.tile([128, 4], f32)
                nc.sync.dma_start(bk_sb[:], d_bk[:])
                bq_sb = const_pool.tile([128, 4], f32)
                nc.sync.dma_start(bq_sb[:], d_bq[:])
                bv_sb = const_pool.tile([1, H * 65], f32r)
                nc.sync.dma_start(bv_sb[:], d_bv[:])
                bp_sb = const_pool.tile([1, E], f32r)
                nc.sync.dma_start(bp_sb[:], d_bp[:])

            # ---- output-projection weights last (consumed last)
            wp_sb = w_pool.tile([128, 2, 8, 512], bf16)
            nc.sync.dma_start(wp_sb[:], d_wp[:])

            # ---- persistent activations
            kT = kqv_pool.tile([128, 4, NTOK], bf16)    # K feature-major
            qT = kqv_pool.tile([128, 4, SHARD], bf16)   # Q feature-major (scaled)
            vA = kqv_pool.tile([128, 5, H * 65], bf16)  # V token-major + ones col
            vTail = kqv_pool.tile([128, NB, H * 65], bf16)  # tail V, 4x replicated
            oT = kqv_pool.tile([128, 8, SHARD], bf16)   # attention out feature-major
            s_half = [kqv_pool.tile([16, TB], f32, name=f"s_half{i}")
                      for i in range(NB)]  # sums
            r_half = [kqv_pool.tile([16, TB], f32, name=f"r_half{i}")
                      for i in range(NB)]  # recips
            # recips rearranged: row 0 = even heads, row 1 = odd heads
            r2 = kqv_pool.tile([2, 16 * TB], f32)

            with tc.tile_pool(name="pp1", bufs=2, space="PSUM") as pp1:
                # K projection: kT[:, m, :] = wk[:, mchunk].T @ xT (+ bk)
                for m in range(4):
                    pa = pp1.tile([128, 512], f32, tag="pa")
                    pb = pp1.tile([128, 32], f32, tag="pb")
                    for e in range(8):
                        nc.tensor.matmul(
                            pa[:], wk_sb[:, m, e, :], xT[:, e, 0:512],
                            start=(e == 0), stop=(e == 7),
                        )
                        nc.tensor.matmul(
                            pb[:], wk_sb[:, m, e, :], xT[:, e, 512:NTOK],
                            start=(e == 0), stop=(e == 7),
                        )
                    if with_bias:
                        nc.scalar.add(kT[:, m, 0:512], pa[:], bk_sb[:, m:m + 1])
                        nc.scalar.add(kT[:, m, 512:NTOK], pb[:], bk_sb[:, m:m + 1])
                    else:
                        nc.scalar.copy(kT[:, m, 0:512], pa[:])
                        nc.scalar.copy(kT[:, m, 512:NTOK], pb[:])

                # Q projection (tokens only, no halo)
                for m in range(4):
                    pa = pp1.tile([128, 512], f32, tag="pa")
                    for e in range(8):
                        nc.tensor.matmul(
                            pa[:], wq_sb[:, m, e, :], xT[:, e, HALO:HALO + SHARD],
                            start=(e == 0), stop=(e == 7),
                        )
                    if with_bias:
                        nc.scalar.add(qT[:, m, :], pa[:], bq_sb[:, m:m + 1])
                    else:
                        nc.scalar.copy(qT[:, m, :], pa[:])

                # V projection token-major (wv pre-augmented with zero ones-cols):
                # vA[tok, h*65:h*65+65] = [x @ Wv_h.T + bv_h | 1]
                tok_sizes = [128, 128, 128, 128, 32]
                for f in range(4):
                    for i in range(5):
                        mt = tok_sizes[i]
                        pa = pp1.tile([128, 260], f32, tag="pv")
                        for e in range(8):
                            nc.tensor.matmul(
                                pa[0:mt, :],
                                xT[:, e, i * 128:i * 128 + mt],
                                wv_sb[:, f, e, :],
                                start=(e == 0), stop=(e == 7 and not with_bias),
                            )
                        if with_bias:
                            nc.tensor.matmul(
                                pa[0:mt, :], ones[0:1, 0:mt],
                                bv_sb[0:1, f * 260:(f + 1) * 260],
                                start=False, stop=True,
                            )
                        nc.vector.tensor_copy(
                            vA[0:mt, i, f * 260:(f + 1) * 260], pa[0:mt, :]
                        )
                        if not with_bias:
                            # ones columns via strided add (psum zeros there)
                            ov_view = vA[0:mt, i, f * 260:(f + 1) * 260].rearrange(
                                "p (h c) -> p h c", c=65
                            )[:, :, 64:65]
                            nc.gpsimd.tensor_scalar_add(ov_view, ov_view, 1.0)

            # replicate the per-block tail-ctx V rows across all four
            # 32-partition groups so packed-tail PV matmuls line up
            for b in range(NB):
                for r in range(4):
                    nc.sync.dma_start(
                        vTail[r * 32:(r + 1) * 32, b, :], vA[0:32, 2 * b + 2, :]
                    )

            # ---- attention
            with (
                tc.tile_pool(name="plgM", bufs=4, space="PSUM") as plgM,
                tc.tile_pool(name="plgT", bufs=2, space="PSUM") as plgT,
                tc.tile_pool(name="pov", bufs=2, space="PSUM") as pov,
            ):
                def emit_recip_chain(b):
                    # gather this block's 16 sum rows from the DRAM scratch,
                    # reciprocate once on DVE, and bounce back through DRAM
                    # into the paired layout (row 0 = even heads, row 1 = odd)
                    nc.sync.dma_start(
                        s_half[b][:],
                        d_ssc[b][0:1, :].rearrange("p (a f) -> (p a) f", a=16),
                    )
                    nc.vector.reciprocal(r_half[b][:], s_half[b][:])
                    nc.sync.dma_start(d_rsc[b][:], r_half[b][:])
                    nc.sync.dma_start(
                        r2[0:2, b * 8 * TB:(b + 1) * 8 * TB].rearrange(
                            "p (a f) -> p a f", f=TB
                        ),
                        d_rsc[b][:].rearrange("(a p) f -> p a f", p=2),
                    )

                prev = None
                for it in range(NB * 4 + 1):
                    if it < NB * 4:
                        b, g = divmod(it, 4)
                        base = b * TB
                        # QK bursts kind-by-kind so each LDWEIGHTS prefetches
                        # under the previous head's matmul
                        lgs = []
                        for hh in range(4):
                            ro = hh * 32
                            qh = qT[ro:ro + 32, g, base:base + TB]
                            lg = plgM.tile([128, 512], f32)
                            nc.tensor.matmul(
                                lg[:, 0:256], kT[ro:ro + 32, g, base:base + 128],
                                qh, start=True, stop=True, tile_position=(ro, 0),
                            )
                            lgs.append(lg)
                        cur_exMs = []
                        for hh in range(4):
                            ro = hh * 32
                            qh = qT[ro:ro + 32, g, base:base + TB]
                            nc.tensor.matmul(
                                lgs[hh][:, 256:512],
                                kT[ro:ro + 32, g, base + 128:base + 256],
                                qh, start=True, stop=True, tile_position=(ro, 0),
                            )
                            nc.vector.tensor_tensor(
                                lgs[hh][:], lgs[hh][:], mask_sb[:, b, 0:512], ADD
                            )
                            ex = exp_pool.tile([128, 512], bf16)
                            nc.scalar.activation(ex[:], lgs[hh][:], EXP)
                            cur_exMs.append(ex)
                        lgt = plgT.tile([128, 512], f32)
                        for hh in range(4):
                            ro = hh * 32
                            qh = qT[ro:ro + 32, g, base:base + TB]
                            nc.tensor.matmul(
                                lgt[ro:ro + 32, 0:256],
                                kT[ro:ro + 32, g, base + 256:base + 288],
                                qh, start=True, stop=True, tile_position=(ro, ro),
                            )
                            nc.tensor.matmul(
                                lgt[ro:ro + 32, 256:512],
                                kbuf_sb[ro:ro + 32, g, :],
                                qh, start=True, stop=True, tile_position=(ro, ro),
                            )
                        nc.vector.tensor_tensor(
                            lgt[:], lgt[:], mask_sb[:, b, 512:1024], ADD
                        )
                        exT = exp_pool.tile([128, 512], bf16)
                        nc.scalar.activation(exT[:], lgt[:], EXP)
                        cur = (b, g, base, cur_exMs, exT)
                    else:
                        cur = None
                    if prev is not None:
                        pb, pg, pbase, pexMs, pexT = prev
                        cur_ov = None
                        for hh in range(4):
                            h = pg * 4 + hh
                            p2, q = divmod(hh, 2)
                            hc = h * 65
                            if q == 0:
                                cur_ov = pov.tile([128, 512], f32)
                            ovr = cur_ov[:, q * 256:q * 256 + 256]
                            nc.tensor.matmul(
                                ovr[0:65, :], vA[:, 2 * pb, hc:hc + 65],
                                pexMs[hh][:, 0:256], start=True, stop=False,
                            )
                            nc.tensor.matmul(
                                ovr[0:65, :], vA[:, 2 * pb + 1, hc:hc + 65],
                                pexMs[hh][:, 256:512], start=False, stop=False,
                            )
                            ro = hh * 32
                            nc.tensor.matmul(
                                ovr[0:65, :], vTail[ro:ro + 32, pb, hc:hc + 65],
                                pexT[ro:ro + 32, 0:256], start=False, stop=False,
                                tile_position=(ro, 0),
                            )
                            nc.tensor.matmul(
                                ovr[0:65, :], vbuf_sb[ro:ro + 32, hc:hc + 65],
                                pexT[ro:ro + 32, 256:512], start=False, stop=True,
                                tile_position=(ro, 0),
                            )
                            od = oT[(h % 2) * 64:(h % 2) * 64 + 64, h // 2,
                                    pbase:pbase + TB]
                            if hh < 2:
                                nc.scalar.copy(od, ovr[0:64, :])
                            else:
                                nc.vector.tensor_copy(od, ovr[0:64, :])
                            if q == 1:
                                idx = pb * H + (h - 1)
                                sst = s_pool.tile([1, 2 * TB], f32)
                                if hh < 2:
                                    nc.vector.tensor_copy(sst[:], cur_ov[64:65, :])
                                else:
                                    nc.scalar.copy(sst[:], cur_ov[64:65, :])
                                lidx = idx - pb * H
                                nc.sync.dma_start(
                                    d_ssc[pb][0:1, lidx * TB:(lidx + 2) * TB],
                                    sst[:],
                                )
                    prev = cur
                    if it == 4:
                        emit_recip_chain(0)
                emit_recip_chain(1)

            # ---- softmax normalization (rank-2 selector broadcast: one
            # matmul + one multiply covers a head pair) + output projection
            with (
                tc.tile_pool(name="prb", bufs=4, space="PSUM") as prb,
                tc.tile_pool(name="pp3", bufs=2, space="PSUM") as pp3,
            ):
                for b in range(NB):
                    for c in range(8):
                        rb2 = prb.tile([128, TB], f32)
                        nc.tensor.matmul(
                            rb2[:], sel2[:],
                            r2[0:2, (b * 8 + c) * TB:(b * 8 + c + 1) * TB]
                            .bitcast(f32r),
                            start=True, stop=True,
                        )
                        sl = oT[:, c, b * TB:(b + 1) * TB]
                        nc.vector.tensor_tensor(sl, sl, rb2[:], MUL)
                    # out-projection for this block's token chunks
                    # (y[tok, :] = oT.T @ wp (+ bp)) overlaps block b+1's
                    # normalization multiplies
                    for m in (2 * b, 2 * b + 1):
                        for f in range(2):
                            pa = pp3.tile([128, 512], f32)
                            for c in range(8):
                                nc.tensor.matmul(
                                    pa[:], oT[:, c, m * 128:(m + 1) * 128],
                                    wp_sb[:, f, c, :], start=(c == 0),
                                    stop=(c == 7 and not with_bias),
                                )
                            if with_bias:
                                nc.tensor.matmul(
                                    pa[:], ones[0:1, 0:128],
                                    bp_sb[0:1, f * 512:(f + 1) * 512],
                                    start=False, stop=True,
                                )
                            ot = out_pool.tile([128, 512], f32)
                            nc.scalar.copy(ot[:], pa[:])
                            nc.sync.dma_start(
                                d_y[m * 128:(m + 1) * 128,
                                    f * 512:(f + 1) * 512], ot[:]
                            )
    return nc


def _get_runner(with_bias=True):
    key = ("runner", with_bias)
    if key in _CACHE:
        return _CACHE[key]
    import jax
    import concourse.mybir as mybir
    from concourse import bass2jax
    from jax.sharding import Mesh, PartitionSpec
    from jax.experimental.shard_map import shard_map

    nc = _build_nc(with_bias)
    bass2jax.install_neuronx_cc_hook()
    partition_name = nc.partition_id_tensor.name if nc.partition_id_tensor else None
    in_names, out_names, out_avals, out_shapes = [], [], [], []
    for alloc in nc.m.functions[0].allocations:
        if not isinstance(alloc, mybir.MemoryLocationSet):
            continue
        name = alloc.memorylocations[0].name
        if alloc.kind == "ExternalInput":
            if name != partition_name:
                in_names.append(name)
        elif alloc.kind == "ExternalOutput":
            shape = tuple(alloc.tensor_shape)
            dtype = mybir.dt.np(alloc.dtype)
            out_names.append(name)
            out_avals.append(jax.core.ShapedArray(shape, dtype))
            out_shapes.append((shape, dtype))
    n_params = len(in_names)
    n_outs = len(out_avals)
    all_in_names = in_names + out_names + ([partition_name] if partition_name else [])
    donate = tuple(range(n_params, n_params + n_outs))

    def _body(*args):
        operands = list(args)
        if partition_name is not None:
            operands.append(bass2jax.partition_id_tensor())
        outs = bass2jax._bass_exec_p.bind(
            *operands,
            out_avals=tuple(out_avals),
            in_names=tuple(all_in_names),
            out_names=tuple(out_names),
            lowering_input_output_aliases=(),
            sim_require_finite=True,
            sim_require_nnan=True,
            nc=nc,
        )
        return tuple(outs)

    devices = jax.devices()[:N_CORES]
    mesh = Mesh(np.asarray(devices), ("core",))
    sharded = jax.jit(
        shard_map(
            _body, mesh=mesh,
            in_specs=(PartitionSpec("core"),) * (n_params + n_outs),
            out_specs=(PartitionSpec("core"),) * n_outs,
            check_rep=False,
        ),
        donate_argnums=donate,
        keep_unused=True,
    )

    def run(in_maps):
        per_core = [[np.asarray(m[name]) for name in in_names] for m in in_maps]
        concat_in = [
            np.concatenate([per_core[c][i] for c in range(N_CORES)], axis=0)
            for i in range(n_params)
        ]
        concat_zeros = [
            np.zeros((N_CORES * s[0], *s[1:]), d) for (s, d) in out_shapes
        ]
        out_arrs = sharded(*concat_in, *concat_zeros)
        return [
            {
                name: np.asarray(out_arrs[i]).reshape(N_CORES, *out_shapes[i][0])[c]
                for i, name in enumerate(out_names)
            }
            for c in range(N_CORES)
        ]

    _CACHE[key] = run
    return run


# ------------------------------------------------------------------- host
def _prep_inputs(x, Wkv, bkv, Wq, bq, Wp, bp, buffer, sample_lengths):
    import ml_dtypes

    bfl = ml_dtypes.bfloat16
    x = np.asarray(x, np.float32)
    Wkv = np.asarray(Wkv, np.float32)
    bkv = np.asarray(bkv, np.float32)
    Wq = np.asarray(Wq, np.float32)
    bq = np.asarray(bq, np.float32)
    Wp = np.asarray(Wp, np.float32)
    bp = np.asarray(bp, np.float32)
    buffer = np.asarray(buffer, np.float32)
    lengths = np.asarray(sample_lengths).astype(np.int64)

    scale = 1.0 / math.sqrt(DK)
    starts = np.concatenate([[0], np.cumsum(lengths)[:-1]]).astype(np.int64)
    t = np.arange(T)
    seg = np.searchsorted(starts, t, side="right") - 1
    j = t - starts[seg]

    # weights pre-rearranged into exact SBUF layouts ([p, ...] partition-major)
    wkT = np.ascontiguousarray(Wkv[:KD, :].T)                       # [E, KD]
    wk_h = wkT.reshape(8, 128, 4, 128).transpose(1, 2, 0, 3).astype(bfl)
    wqT = np.ascontiguousarray(Wq.T * scale)                        # [E, KD]
    wq_h = wqT.reshape(8, 128, 4, 128).transpose(1, 2, 0, 3).astype(bfl)
    wv_aug = np.zeros((E, H, 65), np.float32)
    wv_aug[:, :, :64] = Wkv[KD:, :].T.reshape(E, H, DV)
    wv_h = (
        wv_aug.reshape(E, H * 65).reshape(8, 128, 4, 260)
        .transpose(1, 2, 0, 3).astype(bfl)
    )
    wpT = np.ascontiguousarray(Wp.T)                                # [E, E]
    wp_h = wpT.reshape(8, 128, 2, 512).transpose(1, 2, 0, 3).astype(bfl)

    bk2 = np.ascontiguousarray(bkv[:KD].reshape(4, 128).T)
    bq2 = np.ascontiguousarray((bq * scale).reshape(4, 128).T)
    bv_aug = np.zeros((H, 65), np.float32)
    bv_aug[:, :64] = bkv[KD:].reshape(H, DV)
    bv_aug[:, 64] = 1.0
    bv_row = np.ascontiguousarray(bv_aug.reshape(1, H * 65))
    bp_row = np.ascontiguousarray(bp[None, :])
    ones_row = np.ones((1, 128), np.float32)
    sel2 = np.zeros((2, 128), np.float32)
    sel2[0, :64] = 1.0
    sel2[1, 64:] = 1.0

    kbufT = np.zeros((KD, 32), np.float32)
    kbufT[:, :HALO] = buffer[:, :KD].T
    kbuf_h = kbufT.reshape(4, 128, 32).transpose(1, 0, 2).astype(bfl)
    vbuf = np.zeros((32, H * 65), np.float32)
    vb = vbuf.reshape(32, H, 65)
    vb[:HALO, :, :64] = buffer[:, KD:].reshape(HALO, H, DV)
    vb[:HALO, :, 64] = 1.0
    vbuf4_h = np.tile(vbuf, (4, 1)).astype(bfl)

    xTp = np.zeros((E, T + HALO + 33), np.float32)
    xTp[:, HALO:HALO + T] = x.T

    in_maps = []
    for c in range(N_CORES):
        t0 = c * SHARD
        xT_c = np.ascontiguousarray(
            xTp[:, t0:t0 + NTOK].reshape(8, 128, NTOK).transpose(1, 0, 2)
        ).astype(bfl)
        mask = np.full((128, NB, 1024), NEG, np.float32)
        for bblk in range(NB):
            i = np.arange(TB)
            tt = t0 + bblk * TB + i
            st = starts[seg[tt]]
            jj = j[tt]
            for r in range(2):
                p = np.arange(128)[:, None]
                g = t0 - HALO + bblk * TB + r * 128 + p
                valid = (
                    (g >= tt[None, :] - HALO) & (g <= tt[None, :])
                    & (g >= st[None, :]) & (g >= 0) & (g < T)
                )
                mask[:, bblk, r * 256:(r + 1) * 256] = np.where(valid, 0.0, NEG)
            p32 = np.arange(32)[:, None]
            g = t0 - HALO + bblk * TB + 256 + p32
            valid = (
                (g >= tt[None, :] - HALO) & (g <= tt[None, :])
                & (g >= st[None, :]) & (g >= 0) & (g < T)
            )
            tailm = np.where(valid, 0.0, NEG)
            pb = np.arange(32)[:, None]
            validb = (pb >= jj[None, :]) & (pb <= HALO - 1)
            bufm = np.where(validb, 0.0, NEG)
            for rr in range(4):
                mask[rr * 32:(rr + 1) * 32, bblk, 512:768] = tailm
                mask[rr * 32:(rr + 1) * 32, bblk, 768:1024] = bufm
        in_maps.append({
            "xT": xT_c, "wk": wk_h, "wq": wq_h, "wv": wv_h, "wp": wp_h,
            "kbufT": kbuf_h, "vbuf4": vbuf4_h, "ones": ones_row, "sel2": sel2,
            "mask": np.ascontiguousarray(mask),
            "bk": bk2, "bq": bq2, "bv": bv_row, "bp": bp_row,
        })
    return in_maps, seg, j


def kernel(x, Wkv, bkv, Wq, bq, Wp, bp, buffer, sample_lengths):
    in_maps, seg, j = _prep_inputs(
        x, Wkv, bkv, Wq, bq, Wp, bp, buffer, sample_lengths
    )
    with_bias = bool(
        np.any(np.asarray(bkv)) or np.any(np.asarray(bq)) or np.any(np.asarray(bp))
    )
    run = _get_runner(with_bias)
    results = run(in_maps)
    out_full = np.concatenate([results[c]["yout"] for c in range(N_CORES)], axis=0)
    y = np.zeros((B, MAXL, E), np.float32)
    ok = j < MAXL
    y[seg[ok], j[ok]] = out_full[ok]
    return y
